# revision 40
# baseline (speedup 1.0000x reference)
"""BERT self-attention (B=8, S=1024, D=1024, H=16, Dh=64) on 8 NeuronCores.

Sharding: pure data parallel — core b handles batch element b (B == n_cores),
qkv_weight replicated. No collectives.

Per-core dataflow (all matmuls bf16 with fp32 PSUM accumulation):
  1. X [S,D] loaded first (prefetched 4 deep), cast to bf16 (DVE),
     PE-transposed into X^T [D,S] in groups of 4 chunks per PSUM unload;
     unloads alternate between DVE and ACT (idle early).
  2. W_v loaded+cast up front as [128, kt, 1024]; V computed into 2-bank
     [128,1024] PSUM tiles with stationary X^T chunks (128 matmuls), laid
     out as V' [S, H*(Dh+1)] where each head's 65th column carries
     exp(mask): softmax(s + m) == exp(s)*exp(m) normalized, so the additive
     mask is an exact per-key row scaling of V', and the extra column makes
     the PV matmul emit softmax denominators for free.
  3. Per head pair: W_q/W_k column slices loaded one pair ahead, Q^T,K^T
     computed as [features, S] into 2-bank PSUM tiles (one DVE unload each).
  4. Scores run PAIR-INTERLEAVED: head a (Q^T/K^T rows 0:64, PE row group
     h0) and head b (rows 64:128, row group h64) alternate matmuls per
     k-chunk, so the two 64-contraction matmuls execute CONCURRENTLY in
     disjoint halves of the 128x128 PE array (~1.9x on the scores stage).
     ACT computes exp(0.125*s) PSUM->SBUF(bf16) per [128,1024] tile.
  5. ctx'^T [65,S_q] = V'.T @ expS^T per head; copied to SBUF bf16 (DVE),
     PE-transposed (bf16) back to [S_q,65] four chunks per PSUM tile, one
     strided reciprocal per 4 denominators, cols 0..63 scaled by 1/col64
     on DVE, keeping ACT exp-only.
  6. ctx assembled [S, D] fp32, DMA'd out in column groups as head groups
     complete via the Pool-engine SWDGE queue (parallel to the input loads
     on the SP/ACT HWDGE queues); the final columns leave as two row-half
     DMAs on different queues, the first fired as soon as the qn0
     normalizations land.

Scores concurrency detail: each scores psA tile holds one qn half of BOTH
heads ([a | b]) and ONE 1024-wide exp drains it into a [p, head, s] pair
tile, so all four matmuls of a unit become schedule-ready at the same
instant — the Tile scheduler then places the h0/h64 matmuls back-to-back
(measured 4 ns apart on HW) — and ACT pays half the per-instruction
PSUM-access overhead of split 512-wide exps.

Schedule: exp-score pair tiles are a 2-deep ring of pair-sets. Per pair hp, 8 scores
units (a,b interleaved per k) with fillers woven between: PV(2hp-2) both
halves ride unit 0 (they free the es set that head 2hp+1 overwrites, and
keep ACT saturated instead of idling through a separate pre-pair block —
the unit-0 b-head exps wait only on PV(2hp-2) qn1's first k-read, which
the scheduler orders ahead of them), then PV(2hp-1) halves, QK chunks for
pair hp+1, ctxT of heads 2hp-3 / 2hp-2, and the pair hp+2 weight loads
(DMA at pair start, DVE cast at pair end to keep the strict-FIFO DVE
queue from stalling on the DMA semaphore). Input DMA is spread over both
HWDGE queues: X then W_v on the SP ring, masks + pair-0/1 W on ACT.

No max-subtraction in softmax: scores*scale is bounded (|x| <~ 4 for this
problem's scale) and exp runs in fp32 on ACT.
"""

import sys

import numpy as np

_REPO = "/opt/trn_rl_repo"
if _REPO not in sys.path:
    sys.path.insert(0, _REPO)

B, S, D, H, DH = 8, 1024, 1024, 16, 64
P = 128
NS = S // P          # seq tiles
NK = D // P          # contraction tiles
NHP = H // 2         # head pairs
NQ = 2               # 512-wide S_q chunks
QC = S // NQ         # 512
SCALE = 1.0 / 8.0    # 1/sqrt(DH)
VW = DH + 1          # V' live width per head (extra denominator column)
VP = DH + 2          # V' stored stride per head (pad for 4B-aligned slices)

_NC_CACHE = {}


def _build_nc():
    import concourse.bass as bass
    import concourse.tile as tile
    from concourse import bacc, mybir
    from concourse.masks import make_identity
    from contextlib import ExitStack

    f32 = mybir.dt.float32
    bf16 = mybir.dt.bfloat16
    Exp = mybir.ActivationFunctionType.Exp

    nc = bacc.Bacc("TRN2", target_bir_lowering=False, debug=False)
    x_d = nc.declare_dram_parameter("x", [S, D], f32, isOutput=False)
    w_d = nc.declare_dram_parameter("w", [D, 3 * D], f32, isOutput=False)
    m_d = nc.declare_dram_parameter("m", [S], f32, isOutput=False)
    o_d = nc.declare_dram_parameter("o", [S, D], f32, isOutput=True)

    with tile.TileContext(nc) as tc, ExitStack() as es:
        const = es.enter_context(tc.tile_pool(name="const", bufs=1))
        maskp = es.enter_context(tc.tile_pool(name="maskp", bufs=NS))
        xtp = es.enter_context(tc.tile_pool(name="xtp", bufs=1))
        vp = es.enter_context(tc.tile_pool(name="vp", bufs=NS))
        ctxp = es.enter_context(tc.tile_pool(name="ctxp", bufs=1))
        xstage = es.enter_context(tc.tile_pool(name="xstage", bufs=2))
        wvp = es.enter_context(tc.tile_pool(name="wvp", bufs=1))
        wstage = es.enter_context(tc.tile_pool(name="wstage", bufs=4))
        wqkp = es.enter_context(tc.tile_pool(name="wqkp", bufs=4))
        qktp = es.enter_context(tc.tile_pool(name="qktp", bufs=2))
        esp = es.enter_context(tc.tile_pool(name="esp", bufs=3 * NK))
        ctp = es.enter_context(tc.tile_pool(name="ctp", bufs=4))
        smallp = es.enter_context(tc.tile_pool(name="smallp", bufs=8))
        psA = es.enter_context(tc.tile_pool(name="psA", bufs=2, space="PSUM"))
        psB = es.enter_context(tc.tile_pool(name="psB", bufs=2, space="PSUM"))
        psC = es.enter_context(tc.tile_pool(name="psC", bufs=2, space="PSUM"))

        id_bf = const.tile([P, P], bf16, name="id_bf")
        make_identity(nc, id_bf)
        ones16 = const.tile([P, H], bf16, name="ones16")
        nc.vector.memset(ones16, 1.0)

        # persistent tensors
        xt = xtp.tile([P, NK, S], bf16, name="xt")  # X^T: [d-part, kt, s]
        v_sb = [vp.tile([P, H * VP], bf16, name=f"v{st}", tag="v") for st in range(NS)]
        ctx_all = ctxp.tile([P, NS, D], f32, name="ctx_all")
        ctx_sb = [ctx_all[:, st, :] for st in range(NS)]

        # X loads first on the SP ring (startup critical path); pair-0 W
        # slices after the first four tiles so QK0 can interleave with X^T
        xfs = []
        for i in range(NS):
            xf = xstage.tile([P, D], f32, name=f"xf{i}", tag="xf", bufs=4)
            nc.sync.dma_start(
                out=xf[:, 0:QC], in_=x_d[i * P:(i + 1) * P, 0:QC]
            )
            nc.sync.dma_start(
                out=xf[:, QC:D], in_=x_d[i * P:(i + 1) * P, QC:D]
            )
            xfs.append(xf)

        # All ACT-ring DMA triggers fire BEFORE any ACT compute is queued
        # (strict FIFO): masks, pair-0/1 W slices, then W_v — a DMA queue
        # parallel to the X load on the SP ring.
        msks = []
        for st in range(NS):
            msk = maskp.tile([P, 1], f32, name=f"msk{st}", tag="msk")
            nc.scalar.dma_start(
                out=msk,
                in_=m_d[st * P:(st + 1) * P].rearrange("(p o) -> p o", o=1),
            )
            msks.append(msk)

        # X cast + PE transpose; PSUM unloads in groups of 4 chunks,
        # alternating DVE / ACT
        def emit_xt(i):
            xb = xstage.tile([P, D], bf16, name=f"xb{i}", tag="xb", bufs=2)
            for g in range(2):
                nc.vector.tensor_copy(
                    xb[:, g * QC:(g + 1) * QC], xfs[i][:, g * QC:(g + 1) * QC]
                )
                pst = psB.tile([P, 4 * P], bf16, name=f"px{i}_{g}", tag="psB")
                for c in range(4):
                    j = 4 * g + c
                    nc.tensor.transpose(
                        pst[:, c * P:(c + 1) * P], xb[:, j * P:(j + 1) * P], id_bf
                    )
                eng = nc.vector if (2 * i + g) % 2 else nc.scalar
                dst = xt[:, 4 * g:4 * g + 4, i * P:(i + 1) * P]
                srcp = pst.rearrange("p (c q) -> p c q", c=4)
                if eng is nc.vector:
                    eng.tensor_copy(dst, srcp)
                else:
                    eng.copy(dst, srcp)

        def qk_load_dma(hp, eng=None):
            # W_q/W_k column slice DMAs for this head pair (SP ring by
            # default; startup loads ride the ACT ring so they do not queue
            # behind the 4MB X load on the SP ring's DMA queue)
            wfs = []
            for t, base in enumerate((hp * P, D + hp * P)):
                wf = wstage.tile([P, NK, P], f32, name=f"wf{hp}_{t}", tag="wf")
                (eng or nc.sync).dma_start(
                    out=wf,
                    in_=w_d[:, base:base + P].rearrange("(kt p) c -> p kt c", p=P),
                )
                wfs.append(wf)
            return wfs

        def qk_cast(hp, wfs):
            # bf16 casts (DVE) — issued well after the DMA so the strict-FIFO
            # DVE queue never stalls on the DMA semaphore
            wbf = []
            for t, wf in enumerate(wfs):
                wb = wqkp.tile([P, NK, P], bf16, name=f"wb{hp}_{t}", tag="wb")
                nc.vector.tensor_copy(wb, wf)
                wbf.append(wb)
            return wbf

        def qk_load(hp):
            return qk_cast(hp, qk_load_dma(hp))

        # pair-0/1 W DMAs on the ACT ring (queued right behind the tiny
        # mask DMAs: W0 lands ~t5us, long before QK0 needs it)
        wfs0 = qk_load_dma(0, eng=nc.scalar)
        wfs1 = qk_load_dma(1, eng=nc.scalar)

        # W_v loads via Pool-SWDGE casting DMAs straight into bf16 SBUF —
        # no staging tiles, no DVE casts, no DMA-trigger/cast slot chain
        # (the chain paced wvb to ~t46us and stalled the pair-0 V' matmuls)
        wvb = wvp.tile([P, NK, D], bf16, name="wvb")
        for q in range(4):
            nc.gpsimd.dma_start(
                out=wvb[:, 2 * q:2 * q + 2, :],
                in_=w_d[2 * q * P:(2 * q + 2) * P, 2 * D:3 * D].rearrange(
                    "(kt p) c -> p kt c", p=P
                ),
            )

        # exp(mask) per seq tile — the first ACT compute in the queue
        em = []
        for st in range(NS):
            emt = maskp.tile([P, 1], f32, name=f"em{st}", tag="em")
            nc.scalar.activation(emt, msks[st], Exp)
            em.append(emt)

        # X^T for the first four tiles, then the pair-0 W cast
        for i in range(4):
            emit_xt(i)
        wbf0 = qk_cast(0, wfs0)

        # V' denominator columns = exp(mask) per key row (Pool engine)
        for st in range(NS):
            vcols = v_sb[st].rearrange("p (h c) -> p h c", h=H)[:, :, DH]
            nc.scalar.mul(vcols, ones16, em[st])

        def qk_chunks(hp, wbf):
            # QK as 8 four-matmul chunks into 1-bank psB tiles; each
            # (wsel, n) group is two chunks + a DVE unload, interleavable
            # between score units
            qt_t = qktp.tile([P, S], bf16, name=f"qt{hp}", tag="qt")
            kt_t = qktp.tile([P, S], bf16, name=f"kt{hp}", tag="kt")
            chunks = []
            for wsel, dest in ((1, kt_t), (0, qt_t)):
                for n in range(NQ):
                    cell = {}

                    def c0(cell=cell, wsel=wsel, n=n):
                        ps = psB.tile(
                            [P, QC], f32, name=f"pq{hp}_{wsel}_{n}", tag="psB"
                        )
                        cell["ps"] = ps
                        for k in range(4):
                            nc.tensor.matmul(
                                ps,
                                wbf[wsel][:, k, :],
                                xt[:, k, n * QC:(n + 1) * QC],
                                start=(k == 0),
                                stop=False,
                            )

                    def c1(cell=cell, wsel=wsel, n=n, dest=dest):
                        ps = cell["ps"]
                        for k in range(4, NK):
                            nc.tensor.matmul(
                                ps,
                                wbf[wsel][:, k, :],
                                xt[:, k, n * QC:(n + 1) * QC],
                                start=False,
                                stop=(k == NK - 1),
                            )
                        nc.vector.tensor_copy(
                            dest[:, n * QC:(n + 1) * QC], ps
                        )

                    chunks.append(c0)
                    chunks.append(c1)
            return qt_t, kt_t, chunks

        def emit_v_st(st):
            # V' [S, H*(Dh+2) padded]: stationary X^T chunks, 512-wide W_v;
            # per-key exp(mask) row scaling on the Pool engine
            for half in range(2):
                ps = psB.tile([P, QC], f32, name=f"pv{st}_{half}", tag="psB")
                for k in range(NK):
                    nc.tensor.matmul(
                        ps,
                        xt[:, k, st * P:(st + 1) * P],
                        wvb[:, k, half * QC:(half + 1) * QC],
                        start=(k == 0),
                        stop=(k == NK - 1),
                    )
                vdst = v_sb[st].rearrange("p (h c) -> p h c", h=H)[
                    :, half * 8:(half + 1) * 8, 0:DH
                ]
                vsrc = ps.rearrange("p (h c) -> p h c", h=8)
                nc.scalar.mul(vdst, vsrc, em[st])

        def scores_tiles(h):
            return [
                esp.tile([P, S], bf16, name=f"e{h}_{k}", tag="es") for k in range(NK)
            ]

        def emit_scores_unit(hp, k, esa, esb_, qt_t, kt_t):
            # one k-chunk of BOTH heads of the pair. Each psA tile holds one
            # qn half of BOTH heads ([a | b]), so all four matmuls of a unit
            # become schedule-ready together (the previous unit's exps free
            # both halves at once) and the a/b matmuls run CONCURRENTLY in
            # disjoint PE row groups (a: Q^T/K^T rows 0:64 / row group h0,
            # b: rows 64:128 / h64).
            for qn in range(NQ):
                ps = psA.tile([P, S], f32, name=f"s{hp}_{k}_{qn}", tag="psA")
                nc.tensor.matmul(
                    ps[:, 0:QC],
                    kt_t[0:DH, k * P:(k + 1) * P],
                    qt_t[0:DH, qn * QC:(qn + 1) * QC],
                    start=True,
                    stop=True,
                )
                nc.tensor.matmul(
                    ps[:, QC:S],
                    kt_t[DH:P, k * P:(k + 1) * P],
                    qt_t[DH:P, qn * QC:(qn + 1) * QC],
                    start=True,
                    stop=True,
                )
                nc.scalar.activation(
                    esa[k][:, qn * QC:(qn + 1) * QC], ps[:, 0:QC],
                    Exp, scale=SCALE,
                )
                nc.scalar.activation(
                    esb_[k][:, qn * QC:(qn + 1) * QC], ps[:, QC:S],
                    Exp, scale=SCALE,
                )

        def emit_pv_half(h, esb_, qn):
            # ctx'^T [65, S_q] = V'.T @ expS^T; SBUF bf16 copy (DVE)
            psc = psC.tile([VW, QC], f32, name=f"c{h}_{qn}", tag="psC")
            for k in range(NK):
                nc.tensor.matmul(
                    psc,
                    v_sb[k][:, h * VP:h * VP + VW],
                    esb_[k][:, qn * QC:(qn + 1) * QC],
                    start=(k == 0),
                    stop=(k == NK - 1),
                )
            ct = ctp.tile([VW, QC], bf16, name=f"ct{h}_{qn}", tag="ct")
            nc.vector.tensor_copy(ct, psc)
            return ct

        def emit_ctxt_qn(h, ct, qn, split_muls=False):
            # 4 bf16 PE transposes per PSUM tile back to [S_q, 65];
            # one strided reciprocal per 4 denominators; normalize on DVE
            VW2 = VW + 1  # 66: keeps each chunk's PSUM byte offset 4B-aligned
            pst = psB.tile([P, 4 * VW2], bf16, name=f"pt{h}_{qn}", tag="psB")
            for qs in range(QC // P):
                nc.tensor.transpose(
                    pst[:, qs * VW2:qs * VW2 + VW],
                    ct[:, qs * P:(qs + 1) * P],
                    id_bf[0:VW, 0:VW],
                )
            rec = smallp.tile([P, 4], f32, name=f"r{h}_{qn}", tag="rec")
            pst4 = pst.rearrange("p (c w) -> p c w", w=VW2)
            nc.vector.reciprocal(rec, pst4[:, 0:4, DH])
            for qs in range(QC // P):
                qi = qn * (QC // P) + qs
                if split_muls and qs % 2:
                    nc.scalar.mul(
                        ctx_sb[qi][:, h * DH:(h + 1) * DH],
                        pst[:, qs * VW2:qs * VW2 + DH],
                        rec[:, qs:qs + 1],
                    )
                else:
                    nc.vector.tensor_scalar_mul(
                        ctx_sb[qi][:, h * DH:(h + 1) * DH],
                        pst[:, qs * VW2:qs * VW2 + DH],
                        rec[:, qs:qs + 1],
                    )

        def emit_out_cols(c0, c1, st0=0, st1=NS, eng=None):
            # columns [c0, c1) final for rows [st0*P, st1*P): one 3D DMA.
            # Rides the Pool SWDGE queue by default so output transfers
            # never queue behind the W loads on the SP ring's queue.
            (eng or nc.gpsimd).dma_start(
                out=o_d[st0 * P:st1 * P, c0:c1].rearrange(
                    "(st p) c -> p st c", p=P
                ),
                in_=ctx_all[:, st0:st1, c0:c1],
            )

        # ---- startup: X^T, QK0 (interleaved with the last four X^T
        # tiles: the kt n=0 half only needs X^T columns 0:512)
        qt0 = qktp.tile([P, S], bf16, name="qt0", tag="qt")
        kt0 = qktp.tile([P, S], bf16, name="kt0", tag="kt")
        ps_k = psA.tile([P, S], f32, name="pq0_1", tag="psA")
        ps_q = psA.tile([P, S], f32, name="pq0_0", tag="psA")
        for k in range(NK):
            nc.tensor.matmul(
                ps_k[:, 0:QC], wbf0[1][:, k, :], xt[:, k, 0:QC],
                start=(k == 0), stop=(k == NK - 1),
            )
        emit_xt(4)
        emit_xt(5)
        for k in range(NK):
            nc.tensor.matmul(
                ps_q[:, 0:QC], wbf0[0][:, k, :], xt[:, k, 0:QC],
                start=(k == 0), stop=(k == NK - 1),
            )
        emit_xt(6)
        emit_xt(7)
        for k in range(NK):
            nc.tensor.matmul(
                ps_k[:, QC:S], wbf0[1][:, k, :], xt[:, k, QC:S],
                start=(k == 0), stop=(k == NK - 1),
            )
        nc.vector.tensor_copy(kt0, ps_k)
        for k in range(NK):
            nc.tensor.matmul(
                ps_q[:, QC:S], wbf0[0][:, k, :], xt[:, k, QC:S],
                start=(k == 0), stop=(k == NK - 1),
            )
        nc.vector.tensor_copy(qt0, ps_q)

        # pair-1 W casts first (data already landed), then W_v casts
        wbf1 = []
        for t, wf in enumerate(wfs1):
            wb = wqkp.tile([P, NK, P], bf16, name=f"wb1_{t}", tag="wb")
            nc.vector.tensor_copy(wb, wf)
            wbf1.append(wb)

        # ---- pair 0: interleaved scores(0,1) with QK1 chunks + V' fillers
        # (chunk pairs c0+c1 emitted whole so the psB ring alternates
        # cleanly with the V' tiles — no cross-tile open-group interleave)
        qt1, kt1, chunks1 = qk_chunks(1, wbf1)
        wf2 = qk_load_dma(2)
        es0, es1 = scores_tiles(0), scores_tiles(1)
        for k in range(NK):
            emit_scores_unit(0, k, es0, es1, qt0, kt0)
            if k % 2 == 1:
                chunks1[k - 1]()
                chunks1[k]()
            emit_v_st(k)
        wbf_store = {2: qk_cast(2, wf2)}

        # ---- steady-state pairs 1..7. es sets are a 3-deep ring: head 2hp
        # reuses head 2hp-3's tiles (PV'd mid pair hp-1) and head 2hp+1
        # reuses 2hp-2's, whose PV must therefore run BEFORE this pair's
        # units (the between-block).
        es_sets = {0: es0, 1: es1}
        ct_store = {}
        wf_store = {}
        qt_cur, kt_cur = qt1, kt1
        for hp in range(1, NHP):
            h_even_prev = 2 * hp - 2      # even head of pair hp-1
            h_odd_prev = 2 * hp - 1       # odd head of pair hp-1
            h_odd_prev2 = 2 * hp - 3      # odd head of pair hp-2
            qt_t, kt_t = qt_cur, kt_cur

            # between-block: PV of the even head of the previous pair
            es_even = es_sets.pop(h_even_prev)
            cts_prev = [emit_pv_half(h_even_prev, es_even, 0),
                        emit_pv_half(h_even_prev, es_even, 1)]

            if hp + 1 < NHP:
                qtn, ktn, chunks = qk_chunks(hp + 1, wbf_store.pop(hp + 1))
            else:
                qtn = ktn = None
                chunks = [None] * 8
            if hp + 2 < NHP:
                wf_store[hp + 2] = qk_load_dma(hp + 2)

            es_odd = es_sets.pop(h_odd_prev)
            es_a = scores_tiles(2 * hp)
            es_b = scores_tiles(2 * hp + 1)

            def unit(k, *fillers):
                emit_scores_unit(hp, k, es_a, es_b, qt_t, kt_t)
                for f in fillers:
                    if f is not None:
                        f()

            if hp >= 2:
                ct_p2 = ct_store.pop(h_odd_prev2)
            if hp < NHP - 1:
                unit(0, chunks[0],
                     None if hp < 2 else lambda: emit_ctxt_qn(
                         h_odd_prev2, ct_p2[0], 0))
                unit(1, chunks[1],
                     None if hp < 2 else lambda: emit_ctxt_qn(
                         h_odd_prev2, ct_p2[1], 1))
                unit(2, lambda: ct_store.setdefault(h_odd_prev, []).append(
                    emit_pv_half(h_odd_prev, es_odd, 0)), chunks[2])
                unit(3, chunks[3],
                     lambda: emit_ctxt_qn(h_even_prev, cts_prev[0], 0))
                unit(4, chunks[4],
                     lambda: emit_ctxt_qn(h_even_prev, cts_prev[1], 1))
                unit(5, lambda: ct_store[h_odd_prev].append(
                    emit_pv_half(h_odd_prev, es_odd, 1)), chunks[5])
                unit(6, chunks[6])
                unit(7, chunks[7])
            else:
                # last pair: no QK chunks
                unit(0, lambda: emit_ctxt_qn(h_odd_prev2, ct_p2[0], 0))
                unit(1, lambda: emit_ctxt_qn(h_odd_prev2, ct_p2[1], 1))
                unit(2, lambda: ct_store.setdefault(h_odd_prev, []).append(
                    emit_pv_half(h_odd_prev, es_odd, 0)))
                unit(3, lambda: emit_ctxt_qn(h_even_prev, cts_prev[0], 0))
                unit(4, lambda: emit_ctxt_qn(h_even_prev, cts_prev[1], 1))
                unit(5, lambda: ct_store[h_odd_prev].append(
                    emit_pv_half(h_odd_prev, es_odd, 1)))
                unit(6, None)
                unit(7, None)
            if hp + 2 < NHP:
                wbf_store[hp + 2] = qk_cast(hp + 2, wf_store.pop(hp + 2))
            es_sets[2 * hp] = es_a
            es_sets[2 * hp + 1] = es_b
            qt_cur, kt_cur = qtn, ktn
            if hp == 3:
                emit_out_cols(0, 5 * DH)
            elif hp == 5:
                emit_out_cols(5 * DH, 9 * DH)
            elif hp == 7:
                emit_out_cols(9 * DH, 13 * DH)

        # ---- tail: heads 13 (ctxT), 14, 15 (PV + ctxT). The last columns
        # go out as two row-half DMAs on different rings (parallel queues),
        # the first fired as soon as the qn0 muls land.
        es14 = es_sets.pop(14)
        es15 = es_sets.pop(15)
        ct13 = ct_store.pop(13)
        ct14_0 = emit_pv_half(14, es14, 0)
        emit_ctxt_qn(13, ct13[0], 0)
        ct14_1 = emit_pv_half(14, es14, 1)
        emit_ctxt_qn(14, ct14_0, 0, split_muls=True)
        ct15_0 = emit_pv_half(15, es15, 0)
        emit_ctxt_qn(13, ct13[1], 1)
        ct15_1 = emit_pv_half(15, es15, 1)
        emit_ctxt_qn(15, ct15_0, 0, split_muls=True)
        emit_out_cols(13 * DH, 16 * DH, 0, NS // 2, eng=nc.gpsimd)
        emit_ctxt_qn(14, ct14_1, 1, split_muls=True)
        emit_ctxt_qn(15, ct15_1, 1, split_muls=True)
        emit_out_cols(13 * DH, 16 * DH, NS // 2, NS, eng=nc.scalar)

    nc.finalize()
    return nc


def _get_nc():
    if "nc" not in _NC_CACHE:
        _NC_CACHE["nc"] = _build_nc()
    return _NC_CACHE["nc"]


def _run(hidden_states, attention_mask, qkv_weight, trace=False, **trace_kw):
    from concourse.bass_utils import run_bass_kernel_spmd

    nc = _get_nc()
    hidden = np.ascontiguousarray(np.asarray(hidden_states, dtype=np.float32))
    mask = np.ascontiguousarray(
        np.asarray(attention_mask, dtype=np.float32).reshape(B, S)
    )
    w = np.ascontiguousarray(np.asarray(qkv_weight, dtype=np.float32))
    in_maps = [
        {"x": hidden[b], "w": w, "m": mask[b]} for b in range(B)
    ]
    res = run_bass_kernel_spmd(nc, in_maps, list(range(B)), trace=trace, **trace_kw)
    out = np.stack([np.asarray(res.results[b]["o"]) for b in range(B)], axis=0)
    return out.astype(np.float32), res


def kernel(hidden_states, attention_mask, qkv_weight):
    out, _ = _run(hidden_states, attention_mask, qkv_weight, trace=False)
    return out


if __name__ == "__main__":
    _build_nc()
    print("build ok")


# revision 41
# speedup vs baseline: 1.0813x; 1.0813x over previous
"""BERT self-attention (B=8, S=1024, D=1024, H=16, Dh=64) on 8 NeuronCores.

Sharding: pure data parallel — core b handles batch element b (B == n_cores),
qkv_weight replicated. No collectives.

Per-core dataflow (all matmuls bf16 with fp32 PSUM accumulation):
  1. X [S,D] loaded first (prefetched 4 deep), cast to bf16 (DVE),
     PE-transposed into X^T [D,S] in groups of 4 chunks per PSUM unload;
     unloads alternate between DVE and ACT (idle early).
  2. W_v loaded+cast up front as [128, kt, 1024]; V computed into 2-bank
     [128,1024] PSUM tiles with stationary X^T chunks (128 matmuls), laid
     out as V' [S, H*(Dh+1)] where each head's 65th column carries
     exp(mask): softmax(s + m) == exp(s)*exp(m) normalized, so the additive
     mask is an exact per-key row scaling of V', and the extra column makes
     the PV matmul emit softmax denominators for free.
  3. Per head pair: W_q/W_k column slices loaded one pair ahead, Q^T,K^T
     computed as [features, S] into 2-bank PSUM tiles (one DVE unload each).
  4. Scores run PAIR-INTERLEAVED: head a (Q^T/K^T rows 0:64, PE row group
     h0) and head b (rows 64:128, row group h64) alternate matmuls per
     k-chunk, so the two 64-contraction matmuls execute CONCURRENTLY in
     disjoint halves of the 128x128 PE array (~1.9x on the scores stage).
     ACT computes exp(0.125*s) PSUM->SBUF(bf16) per [128,1024] tile.
  5. ctx'^T [65,S_q] = V'.T @ expS^T per head; copied to SBUF bf16 (DVE),
     PE-transposed (bf16) back to [S_q,65] four chunks per PSUM tile, one
     strided reciprocal per 4 denominators, cols 0..63 scaled by 1/col64
     on DVE, keeping ACT exp-only.
  6. ctx assembled [S, D] fp32, DMA'd out in column groups as head groups
     complete via the Pool-engine SWDGE queue (parallel to the input loads
     on the SP/ACT HWDGE queues); the final columns leave as two row-half
     DMAs on different queues, the first fired as soon as the qn0
     normalizations land.

Scores concurrency detail: each scores psA tile holds one qn half of BOTH
heads ([a | b]) and two 512-wide exps drain it, so all four matmuls of a
unit become schedule-ready at the same instant — the Tile scheduler then
places the h0/h64 matmuls back-to-back (measured 4 ns apart on HW).

Schedule: es tiles are a 3-deep ring of head-sets. Per pair hp, 8 scores
units (a,b interleaved per k) with fillers woven between: PV(2hp-2) both
halves ride unit 0 (they free the es set that head 2hp+1 overwrites, and
keep ACT saturated instead of idling through a separate pre-pair block —
the unit-0 b-head exps wait only on PV(2hp-2) qn1's first k-read, which
the scheduler orders ahead of them), then PV(2hp-1) halves, QK chunks for
pair hp+1, ctxT of heads 2hp-3 / 2hp-2, and the pair hp+2 weight loads
(DMA at pair start, DVE cast at pair end to keep the strict-FIFO DVE
queue from stalling on the DMA semaphore). Input DMA is spread over both
HWDGE queues: X then W_v on the SP ring, masks + pair-0/1 W on ACT.

No max-subtraction in softmax: scores*scale is bounded (|x| <~ 4 for this
problem's scale) and exp runs in fp32 on ACT.
"""

import sys

import numpy as np

_REPO = "/opt/trn_rl_repo"
if _REPO not in sys.path:
    sys.path.insert(0, _REPO)

B, S, D, H, DH = 8, 1024, 1024, 16, 64
P = 128
NS = S // P          # seq tiles
NK = D // P          # contraction tiles
NHP = H // 2         # head pairs
NQ = 2               # 512-wide S_q chunks
QC = S // NQ         # 512
SCALE = 1.0 / 8.0    # 1/sqrt(DH)
VW = DH + 1          # V' live width per head (extra denominator column)
VP = DH + 2          # V' stored stride per head (pad for 4B-aligned slices)

_NC_CACHE = {}


def _build_nc():
    import concourse.bass as bass
    import concourse.tile as tile
    from concourse import bacc, mybir
    from concourse.masks import make_identity
    from contextlib import ExitStack

    f32 = mybir.dt.float32
    bf16 = mybir.dt.bfloat16
    Exp = mybir.ActivationFunctionType.Exp

    nc = bacc.Bacc("TRN2", target_bir_lowering=False, debug=False)
    x_d = nc.declare_dram_parameter("x", [S, D], f32, isOutput=False)
    w_d = nc.declare_dram_parameter("w", [D, 3 * D], f32, isOutput=False)
    m_d = nc.declare_dram_parameter("m", [S], f32, isOutput=False)
    o_d = nc.declare_dram_parameter("o", [S, D], f32, isOutput=True)

    with tile.TileContext(nc) as tc, ExitStack() as es:
        const = es.enter_context(tc.tile_pool(name="const", bufs=1))
        maskp = es.enter_context(tc.tile_pool(name="maskp", bufs=NS))
        xtp = es.enter_context(tc.tile_pool(name="xtp", bufs=1))
        vp = es.enter_context(tc.tile_pool(name="vp", bufs=NS))
        ctxp = es.enter_context(tc.tile_pool(name="ctxp", bufs=1))
        xstage = es.enter_context(tc.tile_pool(name="xstage", bufs=2))
        wvstage = es.enter_context(tc.tile_pool(name="wvstage", bufs=2))
        wvp = es.enter_context(tc.tile_pool(name="wvp", bufs=1))
        wstage = es.enter_context(tc.tile_pool(name="wstage", bufs=4))
        wqkp = es.enter_context(tc.tile_pool(name="wqkp", bufs=4))
        qktp = es.enter_context(tc.tile_pool(name="qktp", bufs=2))
        esp = es.enter_context(tc.tile_pool(name="esp", bufs=3 * NK))
        ctp = es.enter_context(tc.tile_pool(name="ctp", bufs=4))
        smallp = es.enter_context(tc.tile_pool(name="smallp", bufs=8))
        psA = es.enter_context(tc.tile_pool(name="psA", bufs=2, space="PSUM"))
        psB = es.enter_context(tc.tile_pool(name="psB", bufs=2, space="PSUM"))
        psC = es.enter_context(tc.tile_pool(name="psC", bufs=2, space="PSUM"))

        id_bf = const.tile([P, P], bf16, name="id_bf")
        make_identity(nc, id_bf)
        ones16 = const.tile([P, H], bf16, name="ones16")
        nc.vector.memset(ones16, 1.0)

        # persistent tensors
        xt = xtp.tile([P, NK, S], bf16, name="xt")  # X^T: [d-part, kt, s]
        v_sb = [vp.tile([P, H * VP], bf16, name=f"v{st}", tag="v") for st in range(NS)]
        ctx_all = ctxp.tile([P, NS, D], f32, name="ctx_all")
        ctx_sb = [ctx_all[:, st, :] for st in range(NS)]

        # X loads first on the SP ring (startup critical path); pair-0 W
        # slices after the first four tiles so QK0 can interleave with X^T
        xfs = []
        for i in range(NS):
            xf = xstage.tile([P, D], f32, name=f"xf{i}", tag="xf", bufs=4)
            nc.sync.dma_start(
                out=xf[:, 0:QC], in_=x_d[i * P:(i + 1) * P, 0:QC]
            )
            nc.sync.dma_start(
                out=xf[:, QC:D], in_=x_d[i * P:(i + 1) * P, QC:D]
            )
            xfs.append(xf)

        # All ACT-ring DMA triggers fire BEFORE any ACT compute is queued
        # (strict FIFO): masks, pair-0/1 W slices, then W_v — a DMA queue
        # parallel to the X load on the SP ring.
        msks = []
        for st in range(NS):
            msk = maskp.tile([P, 1], f32, name=f"msk{st}", tag="msk")
            nc.scalar.dma_start(
                out=msk,
                in_=m_d[st * P:(st + 1) * P].rearrange("(p o) -> p o", o=1),
            )
            msks.append(msk)

        # X cast + PE transpose; PSUM unloads in groups of 4 chunks,
        # alternating DVE / ACT
        def emit_xt(i):
            xb = xstage.tile([P, D], bf16, name=f"xb{i}", tag="xb", bufs=1)
            for g in range(2):
                nc.vector.tensor_copy(
                    xb[:, g * QC:(g + 1) * QC], xfs[i][:, g * QC:(g + 1) * QC]
                )
                pst = psB.tile([P, 4 * P], bf16, name=f"px{i}_{g}", tag="psB")
                for c in range(4):
                    j = 4 * g + c
                    nc.tensor.transpose(
                        pst[:, c * P:(c + 1) * P], xb[:, j * P:(j + 1) * P], id_bf
                    )
                eng = nc.vector if (2 * i + g) % 2 else nc.scalar
                dst = xt[:, 4 * g:4 * g + 4, i * P:(i + 1) * P]
                srcp = pst.rearrange("p (c q) -> p c q", c=4)
                if eng is nc.vector:
                    eng.tensor_copy(dst, srcp)
                else:
                    eng.copy(dst, srcp)

        def qk_load_dma(hp, eng=None):
            # W_q/W_k column slice DMAs for this head pair (SP ring by
            # default; startup loads ride the ACT ring so they do not queue
            # behind the 4MB X load on the SP ring's DMA queue)
            wfs = []
            for t, base in enumerate((hp * P, D + hp * P)):
                wf = wstage.tile([P, NK, P], f32, name=f"wf{hp}_{t}", tag="wf")
                (eng or nc.sync).dma_start(
                    out=wf,
                    in_=w_d[:, base:base + P].rearrange("(kt p) c -> p kt c", p=P),
                )
                wfs.append(wf)
            return wfs

        def qk_cast(hp, wfs):
            # bf16 casts (DVE) — issued well after the DMA so the strict-FIFO
            # DVE queue never stalls on the DMA semaphore
            wbf = []
            for t, wf in enumerate(wfs):
                wb = wqkp.tile([P, NK, P], bf16, name=f"wb{hp}_{t}", tag="wb")
                nc.vector.tensor_copy(wb, wf)
                wbf.append(wb)
            return wbf

        def qk_load(hp):
            return qk_cast(hp, qk_load_dma(hp))

        # pair-0/1 W DMAs on the ACT ring (queued right behind the tiny
        # mask DMAs: W0 lands ~t5us, long before QK0 needs it)
        wfs0 = qk_load_dma(0, eng=nc.scalar)
        wfs1 = qk_load_dma(1, eng=nc.scalar)

        # W_v full load on the SP ring behind X (lands ~t26us, before the
        # pair-0 V' matmuls need it)
        wvb = wvp.tile([P, NK, D], bf16, name="wvb")
        wvfs = []
        for q in range(4):
            wvf = wvstage.tile([P, 2, D], f32, name=f"wvf{q}", tag="wvf")
            nc.sync.dma_start(
                out=wvf,
                in_=w_d[2 * q * P:(2 * q + 2) * P, 2 * D:3 * D].rearrange(
                    "(kt p) c -> p kt c", p=P
                ),
            )
            wvfs.append(wvf)

        # exp(mask) per seq tile — the first ACT compute in the queue
        em = []
        for st in range(NS):
            emt = maskp.tile([P, 1], f32, name=f"em{st}", tag="em")
            nc.scalar.activation(emt, msks[st], Exp)
            em.append(emt)

        # X^T for the first four tiles, then the pair-0 W cast
        for i in range(4):
            emit_xt(i)
        wbf0 = qk_cast(0, wfs0)

        # V' denominator columns = exp(mask) per key row (Pool engine)
        for st in range(NS):
            vcols = v_sb[st].rearrange("p (h c) -> p h c", h=H)[:, :, DH]
            nc.scalar.mul(vcols, ones16, em[st])

        def qk_chunks(hp, wbf):
            # QK as 8 four-matmul chunks into 1-bank psB tiles; each
            # (wsel, n) group is two chunks + a DVE unload, interleavable
            # between score units
            qt_t = qktp.tile([P, S], bf16, name=f"qt{hp}", tag="qt")
            kt_t = qktp.tile([P, S], bf16, name=f"kt{hp}", tag="kt")
            chunks = []
            for wsel, dest in ((1, kt_t), (0, qt_t)):
                for n in range(NQ):
                    cell = {}

                    def c0(cell=cell, wsel=wsel, n=n):
                        ps = psB.tile(
                            [P, QC], f32, name=f"pq{hp}_{wsel}_{n}", tag="psB"
                        )
                        cell["ps"] = ps
                        for k in range(4):
                            nc.tensor.matmul(
                                ps,
                                wbf[wsel][:, k, :],
                                xt[:, k, n * QC:(n + 1) * QC],
                                start=(k == 0),
                                stop=False,
                            )

                    def c1(cell=cell, wsel=wsel, n=n, dest=dest):
                        ps = cell["ps"]
                        for k in range(4, NK):
                            nc.tensor.matmul(
                                ps,
                                wbf[wsel][:, k, :],
                                xt[:, k, n * QC:(n + 1) * QC],
                                start=False,
                                stop=(k == NK - 1),
                            )
                        nc.vector.tensor_copy(
                            dest[:, n * QC:(n + 1) * QC], ps
                        )

                    chunks.append(c0)
                    chunks.append(c1)
            return qt_t, kt_t, chunks

        def emit_v_st(st):
            # V' [S, H*(Dh+2) padded]: stationary X^T chunks, 512-wide W_v;
            # per-key exp(mask) row scaling on the Pool engine
            for half in range(2):
                ps = psB.tile([P, QC], f32, name=f"pv{st}_{half}", tag="psB")
                for k in range(NK):
                    nc.tensor.matmul(
                        ps,
                        xt[:, k, st * P:(st + 1) * P],
                        wvb[:, k, half * QC:(half + 1) * QC],
                        start=(k == 0),
                        stop=(k == NK - 1),
                    )
                vdst = v_sb[st].rearrange("p (h c) -> p h c", h=H)[
                    :, half * 8:(half + 1) * 8, 0:DH
                ]
                vsrc = ps.rearrange("p (h c) -> p h c", h=8)
                nc.scalar.mul(vdst, vsrc, em[st])

        def scores_tiles(h):
            return [
                esp.tile([P, S], bf16, name=f"e{h}_{k}", tag="es") for k in range(NK)
            ]

        def emit_scores_unit(hp, k, esa, esb_, qt_t, kt_t):
            # one k-chunk of BOTH heads of the pair. Each psA tile holds one
            # qn half of BOTH heads ([a | b]), so all four matmuls of a unit
            # become schedule-ready together (the previous unit's exps free
            # both halves at once) and the a/b matmuls run CONCURRENTLY in
            # disjoint PE row groups (a: Q^T/K^T rows 0:64 / row group h0,
            # b: rows 64:128 / h64).
            for qn in range(NQ):
                ps = psA.tile([P, S], f32, name=f"s{hp}_{k}_{qn}", tag="psA")
                nc.tensor.matmul(
                    ps[:, 0:QC],
                    kt_t[0:DH, k * P:(k + 1) * P],
                    qt_t[0:DH, qn * QC:(qn + 1) * QC],
                    start=True,
                    stop=True,
                )
                nc.tensor.matmul(
                    ps[:, QC:S],
                    kt_t[DH:P, k * P:(k + 1) * P],
                    qt_t[DH:P, qn * QC:(qn + 1) * QC],
                    start=True,
                    stop=True,
                )
                nc.scalar.activation(
                    esa[k][:, qn * QC:(qn + 1) * QC], ps[:, 0:QC],
                    Exp, scale=SCALE,
                )
                nc.scalar.activation(
                    esb_[k][:, qn * QC:(qn + 1) * QC], ps[:, QC:S],
                    Exp, scale=SCALE,
                )

        def emit_pv_half(h, esb_, qn):
            # ctx'^T [65, S_q] = V'.T @ expS^T; SBUF bf16 copy (DVE)
            psc = psC.tile([VW, QC], f32, name=f"c{h}_{qn}", tag="psC")
            for k in range(NK):
                nc.tensor.matmul(
                    psc,
                    v_sb[k][:, h * VP:h * VP + VW],
                    esb_[k][:, qn * QC:(qn + 1) * QC],
                    start=(k == 0),
                    stop=(k == NK - 1),
                )
            ct = ctp.tile([VW, QC], bf16, name=f"ct{h}_{qn}", tag="ct")
            nc.vector.tensor_copy(ct, psc)
            return ct

        def emit_ctxt_qn(h, ct, qn, split_muls=False):
            # 4 bf16 PE transposes per PSUM tile back to [S_q, 65];
            # one strided reciprocal per 4 denominators; normalize on DVE
            VW2 = VW + 1  # 66: keeps each chunk's PSUM byte offset 4B-aligned
            pst = psB.tile([P, 4 * VW2], bf16, name=f"pt{h}_{qn}", tag="psB")
            for qs in range(QC // P):
                nc.tensor.transpose(
                    pst[:, qs * VW2:qs * VW2 + VW],
                    ct[:, qs * P:(qs + 1) * P],
                    id_bf[0:VW, 0:VW],
                )
            rec = smallp.tile([P, 4], f32, name=f"r{h}_{qn}", tag="rec")
            pst4 = pst.rearrange("p (c w) -> p c w", w=VW2)
            nc.vector.reciprocal(rec, pst4[:, 0:4, DH])
            for qs in range(QC // P):
                qi = qn * (QC // P) + qs
                if split_muls and qs % 2:
                    nc.scalar.mul(
                        ctx_sb[qi][:, h * DH:(h + 1) * DH],
                        pst[:, qs * VW2:qs * VW2 + DH],
                        rec[:, qs:qs + 1],
                    )
                else:
                    nc.vector.tensor_scalar_mul(
                        ctx_sb[qi][:, h * DH:(h + 1) * DH],
                        pst[:, qs * VW2:qs * VW2 + DH],
                        rec[:, qs:qs + 1],
                    )

        def emit_out_cols(c0, c1, st0=0, st1=NS, eng=None):
            # columns [c0, c1) final for rows [st0*P, st1*P): one 3D DMA.
            # Rides the Pool SWDGE queue by default so output transfers
            # never queue behind the W loads on the SP ring's queue.
            (eng or nc.gpsimd).dma_start(
                out=o_d[st0 * P:st1 * P, c0:c1].rearrange(
                    "(st p) c -> p st c", p=P
                ),
                in_=ctx_all[:, st0:st1, c0:c1],
            )

        # ---- startup: X^T, QK0 (interleaved with the last four X^T
        # tiles: the kt n=0 half only needs X^T columns 0:512)
        qt0 = qktp.tile([P, S], bf16, name="qt0", tag="qt")
        kt0 = qktp.tile([P, S], bf16, name="kt0", tag="kt")
        ps_k = psA.tile([P, S], f32, name="pq0_1", tag="psA")
        ps_q = psA.tile([P, S], f32, name="pq0_0", tag="psA")
        for k in range(NK):
            nc.tensor.matmul(
                ps_k[:, 0:QC], wbf0[1][:, k, :], xt[:, k, 0:QC],
                start=(k == 0), stop=(k == NK - 1),
            )
        emit_xt(4)
        emit_xt(5)
        for k in range(NK):
            nc.tensor.matmul(
                ps_q[:, 0:QC], wbf0[0][:, k, :], xt[:, k, 0:QC],
                start=(k == 0), stop=(k == NK - 1),
            )
        emit_xt(6)
        emit_xt(7)
        for k in range(NK):
            nc.tensor.matmul(
                ps_k[:, QC:S], wbf0[1][:, k, :], xt[:, k, QC:S],
                start=(k == 0), stop=(k == NK - 1),
            )
        nc.vector.tensor_copy(kt0, ps_k)
        for k in range(NK):
            nc.tensor.matmul(
                ps_q[:, QC:S], wbf0[0][:, k, :], xt[:, k, QC:S],
                start=(k == 0), stop=(k == NK - 1),
            )
        nc.vector.tensor_copy(qt0, ps_q)

        # pair-1 W casts first (data already landed), then W_v casts
        wbf1 = []
        for t, wf in enumerate(wfs1):
            wb = wqkp.tile([P, NK, P], bf16, name=f"wb1_{t}", tag="wb")
            nc.vector.tensor_copy(wb, wf)
            wbf1.append(wb)
        for q in range(4):
            nc.vector.tensor_copy(wvb[:, 2 * q:2 * q + 2, :], wvfs[q])

        # ---- pair 0: interleaved scores(0,1) with QK1 chunks + V' fillers
        # (chunk pairs c0+c1 emitted whole so the psB ring alternates
        # cleanly with the V' tiles — no cross-tile open-group interleave)
        qt1, kt1, chunks1 = qk_chunks(1, wbf1)
        wf2 = qk_load_dma(2)
        es0, es1 = scores_tiles(0), scores_tiles(1)
        for k in range(NK):
            emit_scores_unit(0, k, es0, es1, qt0, kt0)
            if k % 2 == 1:
                chunks1[k - 1]()
                chunks1[k]()
            emit_v_st(k)
        wbf_store = {2: qk_cast(2, wf2)}

        # ---- steady-state pairs 1..7. es sets are a 3-deep ring: head 2hp
        # reuses head 2hp-3's tiles (PV'd mid pair hp-1) and head 2hp+1
        # reuses 2hp-2's, whose PV must therefore run BEFORE this pair's
        # units (the between-block).
        es_sets = {0: es0, 1: es1}
        ct_store = {}
        wf_store = {}
        qt_cur, kt_cur = qt1, kt1
        for hp in range(1, NHP):
            h_even_prev = 2 * hp - 2      # even head of pair hp-1
            h_odd_prev = 2 * hp - 1       # odd head of pair hp-1
            h_odd_prev2 = 2 * hp - 3      # odd head of pair hp-2
            qt_t, kt_t = qt_cur, kt_cur

            # between-block: PV of the even head of the previous pair
            es_even = es_sets.pop(h_even_prev)
            cts_prev = [emit_pv_half(h_even_prev, es_even, 0),
                        emit_pv_half(h_even_prev, es_even, 1)]

            if hp + 1 < NHP:
                qtn, ktn, chunks = qk_chunks(hp + 1, wbf_store.pop(hp + 1))
            else:
                qtn = ktn = None
                chunks = [None] * 8
            if hp + 2 < NHP:
                wf_store[hp + 2] = qk_load_dma(hp + 2)

            es_odd = es_sets.pop(h_odd_prev)
            es_a = scores_tiles(2 * hp)
            es_b = scores_tiles(2 * hp + 1)

            def unit(k, *fillers):
                emit_scores_unit(hp, k, es_a, es_b, qt_t, kt_t)
                for f in fillers:
                    if f is not None:
                        f()

            if hp >= 2:
                ct_p2 = ct_store.pop(h_odd_prev2)
            if hp < NHP - 1:
                unit(0, chunks[0],
                     None if hp < 2 else lambda: emit_ctxt_qn(
                         h_odd_prev2, ct_p2[0], 0))
                unit(1, chunks[1],
                     None if hp < 2 else lambda: emit_ctxt_qn(
                         h_odd_prev2, ct_p2[1], 1))
                unit(2, lambda: ct_store.setdefault(h_odd_prev, []).append(
                    emit_pv_half(h_odd_prev, es_odd, 0)), chunks[2])
                unit(3, chunks[3],
                     lambda: emit_ctxt_qn(h_even_prev, cts_prev[0], 0))
                unit(4, chunks[4],
                     lambda: emit_ctxt_qn(h_even_prev, cts_prev[1], 1))
                unit(5, lambda: ct_store[h_odd_prev].append(
                    emit_pv_half(h_odd_prev, es_odd, 1)), chunks[5])
                unit(6, chunks[6])
                unit(7, chunks[7])
            else:
                # last pair: no QK chunks
                unit(0, lambda: emit_ctxt_qn(h_odd_prev2, ct_p2[0], 0))
                unit(1, lambda: emit_ctxt_qn(h_odd_prev2, ct_p2[1], 1))
                unit(2, lambda: ct_store.setdefault(h_odd_prev, []).append(
                    emit_pv_half(h_odd_prev, es_odd, 0)))
                unit(3, lambda: emit_ctxt_qn(h_even_prev, cts_prev[0], 0))
                unit(4, lambda: emit_ctxt_qn(h_even_prev, cts_prev[1], 1))
                unit(5, lambda: ct_store[h_odd_prev].append(
                    emit_pv_half(h_odd_prev, es_odd, 1)))
                unit(6, None)
                unit(7, None)
            if hp + 2 < NHP:
                wbf_store[hp + 2] = qk_cast(hp + 2, wf_store.pop(hp + 2))
            es_sets[2 * hp] = es_a
            es_sets[2 * hp + 1] = es_b
            qt_cur, kt_cur = qtn, ktn
            if hp == 3:
                emit_out_cols(0, 5 * DH)
            elif hp == 5:
                emit_out_cols(5 * DH, 9 * DH)
            elif hp == 7:
                emit_out_cols(9 * DH, 13 * DH)

        # ---- tail: heads 13 (ctxT), 14, 15 (PV + ctxT). The last columns
        # go out as two row-half DMAs on different rings (parallel queues),
        # the first fired as soon as the qn0 muls land.
        es14 = es_sets.pop(14)
        es15 = es_sets.pop(15)
        ct13 = ct_store.pop(13)
        ct14_0 = emit_pv_half(14, es14, 0)
        emit_ctxt_qn(13, ct13[0], 0)
        ct14_1 = emit_pv_half(14, es14, 1)
        emit_ctxt_qn(14, ct14_0, 0, split_muls=True)
        ct15_0 = emit_pv_half(15, es15, 0)
        emit_ctxt_qn(13, ct13[1], 1)
        ct15_1 = emit_pv_half(15, es15, 1)
        emit_ctxt_qn(15, ct15_0, 0, split_muls=True)
        emit_out_cols(13 * DH, 16 * DH, 0, NS // 2, eng=nc.gpsimd)
        emit_ctxt_qn(14, ct14_1, 1, split_muls=True)
        emit_ctxt_qn(15, ct15_1, 1, split_muls=True)
        emit_out_cols(13 * DH, 16 * DH, NS // 2, NS, eng=nc.scalar)

    nc.finalize()
    return nc


def _get_nc():
    if "nc" not in _NC_CACHE:
        _NC_CACHE["nc"] = _build_nc()
    return _NC_CACHE["nc"]


def _run(hidden_states, attention_mask, qkv_weight, trace=False, **trace_kw):
    from concourse.bass_utils import run_bass_kernel_spmd

    nc = _get_nc()
    hidden = np.ascontiguousarray(np.asarray(hidden_states, dtype=np.float32))
    mask = np.ascontiguousarray(
        np.asarray(attention_mask, dtype=np.float32).reshape(B, S)
    )
    w = np.ascontiguousarray(np.asarray(qkv_weight, dtype=np.float32))
    in_maps = [
        {"x": hidden[b], "w": w, "m": mask[b]} for b in range(B)
    ]
    res = run_bass_kernel_spmd(nc, in_maps, list(range(B)), trace=trace, **trace_kw)
    out = np.stack([np.asarray(res.results[b]["o"]) for b in range(B)], axis=0)
    return out.astype(np.float32), res


def kernel(hidden_states, attention_mask, qkv_weight):
    out, _ = _run(hidden_states, attention_mask, qkv_weight, trace=False)
    return out


if __name__ == "__main__":
    _build_nc()
    print("build ok")


# revision 42
# speedup vs baseline: 1.0958x; 1.0134x over previous
"""BERT self-attention (B=8, S=1024, D=1024, H=16, Dh=64) on 8 NeuronCores.

Sharding: pure data parallel — core b handles batch element b (B == n_cores),
qkv_weight replicated. No collectives.

Per-core dataflow (all matmuls bf16 with fp32 PSUM accumulation):
  1. X [S,D] loaded first (prefetched 4 deep), cast to bf16 (DVE),
     PE-transposed into X^T [D,S] in groups of 4 chunks per PSUM unload;
     unloads alternate between DVE and ACT (idle early).
  2. W_v loaded+cast up front as [128, kt, 1024]; V computed into 2-bank
     [128,1024] PSUM tiles with stationary X^T chunks (128 matmuls), laid
     out as V' [S, H*(Dh+1)] where each head's 65th column carries
     exp(mask): softmax(s + m) == exp(s)*exp(m) normalized, so the additive
     mask is an exact per-key row scaling of V', and the extra column makes
     the PV matmul emit softmax denominators for free.
  3. Per head pair: W_q/W_k column slices loaded one pair ahead, Q^T,K^T
     computed as [features, S] into 2-bank PSUM tiles (one DVE unload each).
  4. Scores run PAIR-INTERLEAVED: head a (Q^T/K^T rows 0:64, PE row group
     h0) and head b (rows 64:128, row group h64) alternate matmuls per
     k-chunk, so the two 64-contraction matmuls execute CONCURRENTLY in
     disjoint halves of the 128x128 PE array (~1.9x on the scores stage).
     ACT computes exp(0.125*s) PSUM->SBUF(bf16) per [128,1024] tile.
  5. ctx'^T [65,S_q] = V'.T @ expS^T per head; copied to SBUF bf16 (DVE),
     PE-transposed (bf16) back to [S_q,65] four chunks per PSUM tile, one
     strided reciprocal per 4 denominators, cols 0..63 scaled by 1/col64
     on DVE, keeping ACT exp-only.
  6. ctx assembled [S, D] fp32, DMA'd out in column groups as head groups
     complete via the Pool-engine SWDGE queue (parallel to the input loads
     on the SP/ACT HWDGE queues); the final columns leave as two row-half
     DMAs on different queues, the first fired as soon as the qn0
     normalizations land.

Scores concurrency detail: each scores psA tile holds one qn half of BOTH
heads ([a | b]) and two 512-wide exps drain it, so all four matmuls of a
unit become schedule-ready at the same instant — the Tile scheduler then
places the h0/h64 matmuls back-to-back (measured 4 ns apart on HW).

Schedule: es tiles are a 3-deep ring of head-sets. Per pair hp, 8 scores
units (a,b interleaved per k) with fillers woven between: PV(2hp-2) both
halves ride unit 0 (they free the es set that head 2hp+1 overwrites, and
keep ACT saturated instead of idling through a separate pre-pair block —
the unit-0 b-head exps wait only on PV(2hp-2) qn1's first k-read, which
the scheduler orders ahead of them), then PV(2hp-1) halves, QK chunks for
pair hp+1, ctxT of heads 2hp-3 / 2hp-2, and the pair hp+2 weight loads
(DMA at pair start, DVE cast at pair end to keep the strict-FIFO DVE
queue from stalling on the DMA semaphore). Input DMA is spread over both
HWDGE queues: X then W_v on the SP ring, masks + pair-0/1 W on ACT.

No max-subtraction in softmax: scores*scale is bounded (|x| <~ 4 for this
problem's scale) and exp runs in fp32 on ACT.
"""

import sys

import numpy as np

_REPO = "/opt/trn_rl_repo"
if _REPO not in sys.path:
    sys.path.insert(0, _REPO)

B, S, D, H, DH = 8, 1024, 1024, 16, 64
P = 128
NS = S // P          # seq tiles
NK = D // P          # contraction tiles
NHP = H // 2         # head pairs
NQ = 2               # 512-wide S_q chunks
QC = S // NQ         # 512
SCALE = 1.0 / 8.0    # 1/sqrt(DH)
VW = DH + 1          # V' live width per head (extra denominator column)
VP = DH + 2          # V' stored stride per head (pad for 4B-aligned slices)

_NC_CACHE = {}


def _build_nc():
    import concourse.bass as bass
    import concourse.tile as tile
    from concourse import bacc, mybir
    from concourse.masks import make_identity
    from contextlib import ExitStack

    f32 = mybir.dt.float32
    bf16 = mybir.dt.bfloat16
    Exp = mybir.ActivationFunctionType.Exp

    nc = bacc.Bacc("TRN2", target_bir_lowering=False, debug=False)
    x_d = nc.declare_dram_parameter("x", [S, D], f32, isOutput=False)
    w_d = nc.declare_dram_parameter("w", [D, 3 * D], f32, isOutput=False)
    m_d = nc.declare_dram_parameter("m", [S], f32, isOutput=False)
    o_d = nc.declare_dram_parameter("o", [S, D], f32, isOutput=True)

    with tile.TileContext(nc) as tc, ExitStack() as es:
        const = es.enter_context(tc.tile_pool(name="const", bufs=1))
        maskp = es.enter_context(tc.tile_pool(name="maskp", bufs=NS))
        xtp = es.enter_context(tc.tile_pool(name="xtp", bufs=1))
        vp = es.enter_context(tc.tile_pool(name="vp", bufs=NS))
        ctxp = es.enter_context(tc.tile_pool(name="ctxp", bufs=1))
        xstage = es.enter_context(tc.tile_pool(name="xstage", bufs=2))
        wvstage = es.enter_context(tc.tile_pool(name="wvstage", bufs=2))
        wvp = es.enter_context(tc.tile_pool(name="wvp", bufs=1))
        wstage = es.enter_context(tc.tile_pool(name="wstage", bufs=4))
        wqkp = es.enter_context(tc.tile_pool(name="wqkp", bufs=4))
        qktp = es.enter_context(tc.tile_pool(name="qktp", bufs=2))
        esp = es.enter_context(tc.tile_pool(name="esp", bufs=3 * NK))
        ctp = es.enter_context(tc.tile_pool(name="ctp", bufs=4))
        smallp = es.enter_context(tc.tile_pool(name="smallp", bufs=8))
        psA = es.enter_context(tc.tile_pool(name="psA", bufs=2, space="PSUM"))
        psB = es.enter_context(tc.tile_pool(name="psB", bufs=2, space="PSUM"))
        psC = es.enter_context(tc.tile_pool(name="psC", bufs=2, space="PSUM"))

        id_bf = const.tile([P, P], bf16, name="id_bf")
        make_identity(nc, id_bf)
        ones16 = const.tile([P, H], bf16, name="ones16")
        nc.vector.memset(ones16, 1.0)

        # persistent tensors
        xt = xtp.tile([P, NK, S], bf16, name="xt")  # X^T: [d-part, kt, s]
        v_sb = [vp.tile([P, H * VP], bf16, name=f"v{st}", tag="v") for st in range(NS)]
        ctx_all = ctxp.tile([P, NS, D], f32, name="ctx_all")
        ctx_sb = [ctx_all[:, st, :] for st in range(NS)]

        # X loads first on the SP ring (startup critical path); pair-0 W
        # slices after the first four tiles so QK0 can interleave with X^T
        xfs = []
        for i in range(NS):
            xf = xstage.tile([P, D], f32, name=f"xf{i}", tag="xf", bufs=4)
            nc.sync.dma_start(
                out=xf[:, 0:QC], in_=x_d[i * P:(i + 1) * P, 0:QC]
            )
            nc.sync.dma_start(
                out=xf[:, QC:D], in_=x_d[i * P:(i + 1) * P, QC:D]
            )
            xfs.append(xf)

        # All ACT-ring DMA triggers fire BEFORE any ACT compute is queued
        # (strict FIFO): masks, pair-0/1 W slices, then W_v — a DMA queue
        # parallel to the X load on the SP ring.
        msks = []
        for st in range(NS):
            msk = maskp.tile([P, 1], f32, name=f"msk{st}", tag="msk")
            nc.scalar.dma_start(
                out=msk,
                in_=m_d[st * P:(st + 1) * P].rearrange("(p o) -> p o", o=1),
            )
            msks.append(msk)

        # X cast + PE transpose; PSUM unloads in groups of 4 chunks,
        # alternating DVE / ACT
        def emit_xt(i):
            xb = xstage.tile([P, D], bf16, name=f"xb{i}", tag="xb", bufs=1)
            for g in range(2):
                nc.vector.tensor_copy(
                    xb[:, g * QC:(g + 1) * QC], xfs[i][:, g * QC:(g + 1) * QC]
                )
                pst = psB.tile([P, 4 * P], bf16, name=f"px{i}_{g}", tag="psB")
                for c in range(4):
                    j = 4 * g + c
                    nc.tensor.transpose(
                        pst[:, c * P:(c + 1) * P], xb[:, j * P:(j + 1) * P], id_bf
                    )
                eng = nc.vector if (2 * i + g) % 2 else nc.scalar
                dst = xt[:, 4 * g:4 * g + 4, i * P:(i + 1) * P]
                srcp = pst.rearrange("p (c q) -> p c q", c=4)
                if eng is nc.vector:
                    eng.tensor_copy(dst, srcp)
                else:
                    eng.copy(dst, srcp)

        def qk_load_dma(hp, eng=None):
            # W_q/W_k column slice DMAs for this head pair (SP ring by
            # default; startup loads ride the ACT ring so they do not queue
            # behind the 4MB X load on the SP ring's DMA queue)
            wfs = []
            for t, base in enumerate((hp * P, D + hp * P)):
                wf = wstage.tile([P, NK, P], f32, name=f"wf{hp}_{t}", tag="wf")
                (eng or nc.sync).dma_start(
                    out=wf,
                    in_=w_d[:, base:base + P].rearrange("(kt p) c -> p kt c", p=P),
                )
                wfs.append(wf)
            return wfs

        def qk_cast(hp, wfs):
            # bf16 casts (DVE) — issued well after the DMA so the strict-FIFO
            # DVE queue never stalls on the DMA semaphore
            wbf = []
            for t, wf in enumerate(wfs):
                wb = wqkp.tile([P, NK, P], bf16, name=f"wb{hp}_{t}", tag="wb")
                nc.vector.tensor_copy(wb, wf)
                wbf.append(wb)
            return wbf

        def qk_load(hp):
            return qk_cast(hp, qk_load_dma(hp))

        # pair-0/1 W DMAs on the ACT ring (queued right behind the tiny
        # mask DMAs: W0 lands ~t5us, long before QK0 needs it)
        wfs0 = qk_load_dma(0, eng=nc.scalar)

        # W_v full load on the SP ring behind X (lands ~t26us, before the
        # pair-0 V' matmuls need it)
        wvb = wvp.tile([P, NK, D], bf16, name="wvb")
        wvfs = []
        for q in range(4):
            wvf = wvstage.tile([P, 2, D], f32, name=f"wvf{q}", tag="wvf")
            nc.sync.dma_start(
                out=wvf,
                in_=w_d[2 * q * P:(2 * q + 2) * P, 2 * D:3 * D].rearrange(
                    "(kt p) c -> p kt c", p=P
                ),
            )
            wvfs.append(wvf)

        # exp(mask) per seq tile — the first ACT compute in the queue
        em = []
        for st in range(NS):
            emt = maskp.tile([P, 1], f32, name=f"em{st}", tag="em")
            nc.scalar.activation(emt, msks[st], Exp)
            em.append(emt)

        # X^T for the first four tiles, then the pair-0 W cast
        for i in range(4):
            emit_xt(i)
        wbf0 = qk_cast(0, wfs0)
        # pair-1 W triggers AFTER the pair-0 cast: their wstage-slot wait
        # (ring of 3, slot freed by that cast) must not block the ACT
        # stream's em/vcols/X^T-unload ops behind it
        wfs1 = qk_load_dma(1, eng=nc.scalar)

        # V' denominator columns = exp(mask) per key row (Pool engine)
        for st in range(NS):
            vcols = v_sb[st].rearrange("p (h c) -> p h c", h=H)[:, :, DH]
            nc.scalar.mul(vcols, ones16, em[st])

        def qk_chunks(hp, wbf):
            # QK as 8 four-matmul chunks into 1-bank psB tiles; each
            # (wsel, n) group is two chunks + a DVE unload, interleavable
            # between score units
            qt_t = qktp.tile([P, S], bf16, name=f"qt{hp}", tag="qt")
            kt_t = qktp.tile([P, S], bf16, name=f"kt{hp}", tag="kt")
            chunks = []
            for wsel, dest in ((1, kt_t), (0, qt_t)):
                for n in range(NQ):
                    cell = {}

                    def c0(cell=cell, wsel=wsel, n=n):
                        ps = psB.tile(
                            [P, QC], f32, name=f"pq{hp}_{wsel}_{n}", tag="psB"
                        )
                        cell["ps"] = ps
                        for k in range(4):
                            nc.tensor.matmul(
                                ps,
                                wbf[wsel][:, k, :],
                                xt[:, k, n * QC:(n + 1) * QC],
                                start=(k == 0),
                                stop=False,
                            )

                    def c1(cell=cell, wsel=wsel, n=n, dest=dest):
                        ps = cell["ps"]
                        for k in range(4, NK):
                            nc.tensor.matmul(
                                ps,
                                wbf[wsel][:, k, :],
                                xt[:, k, n * QC:(n + 1) * QC],
                                start=False,
                                stop=(k == NK - 1),
                            )
                        nc.vector.tensor_copy(
                            dest[:, n * QC:(n + 1) * QC], ps
                        )

                    chunks.append(c0)
                    chunks.append(c1)
            return qt_t, kt_t, chunks

        def emit_v_st(st):
            # V' [S, H*(Dh+2) padded]: stationary X^T chunks, 512-wide W_v;
            # per-key exp(mask) row scaling on the Pool engine
            for half in range(2):
                ps = psB.tile([P, QC], f32, name=f"pv{st}_{half}", tag="psB")
                for k in range(NK):
                    nc.tensor.matmul(
                        ps,
                        xt[:, k, st * P:(st + 1) * P],
                        wvb[:, k, half * QC:(half + 1) * QC],
                        start=(k == 0),
                        stop=(k == NK - 1),
                    )
                vdst = v_sb[st].rearrange("p (h c) -> p h c", h=H)[
                    :, half * 8:(half + 1) * 8, 0:DH
                ]
                vsrc = ps.rearrange("p (h c) -> p h c", h=8)
                nc.scalar.mul(vdst, vsrc, em[st])

        def scores_tiles(h):
            return [
                esp.tile([P, S], bf16, name=f"e{h}_{k}", tag="es") for k in range(NK)
            ]

        def emit_scores_unit(hp, k, esa, esb_, qt_t, kt_t):
            # one k-chunk of BOTH heads of the pair. Each psA tile holds one
            # qn half of BOTH heads ([a | b]), so all four matmuls of a unit
            # become schedule-ready together (the previous unit's exps free
            # both halves at once) and the a/b matmuls run CONCURRENTLY in
            # disjoint PE row groups (a: Q^T/K^T rows 0:64 / row group h0,
            # b: rows 64:128 / h64).
            for qn in range(NQ):
                ps = psA.tile([P, S], f32, name=f"s{hp}_{k}_{qn}", tag="psA")
                nc.tensor.matmul(
                    ps[:, 0:QC],
                    kt_t[0:DH, k * P:(k + 1) * P],
                    qt_t[0:DH, qn * QC:(qn + 1) * QC],
                    start=True,
                    stop=True,
                )
                nc.tensor.matmul(
                    ps[:, QC:S],
                    kt_t[DH:P, k * P:(k + 1) * P],
                    qt_t[DH:P, qn * QC:(qn + 1) * QC],
                    start=True,
                    stop=True,
                )
                nc.scalar.activation(
                    esa[k][:, qn * QC:(qn + 1) * QC], ps[:, 0:QC],
                    Exp, scale=SCALE,
                )
                nc.scalar.activation(
                    esb_[k][:, qn * QC:(qn + 1) * QC], ps[:, QC:S],
                    Exp, scale=SCALE,
                )

        def emit_pv_half(h, esb_, qn):
            # ctx'^T [65, S_q] = V'.T @ expS^T; SBUF bf16 copy (DVE)
            psc = psC.tile([VW, QC], f32, name=f"c{h}_{qn}", tag="psC")
            for k in range(NK):
                nc.tensor.matmul(
                    psc,
                    v_sb[k][:, h * VP:h * VP + VW],
                    esb_[k][:, qn * QC:(qn + 1) * QC],
                    start=(k == 0),
                    stop=(k == NK - 1),
                )
            ct = ctp.tile([VW, QC], bf16, name=f"ct{h}_{qn}", tag="ct")
            nc.vector.tensor_copy(ct, psc)
            return ct

        def emit_ctxt_qn(h, ct, qn, split_muls=False):
            # 4 bf16 PE transposes per PSUM tile back to [S_q, 65];
            # one strided reciprocal per 4 denominators; normalize on DVE
            VW2 = VW + 1  # 66: keeps each chunk's PSUM byte offset 4B-aligned
            pst = psB.tile([P, 4 * VW2], bf16, name=f"pt{h}_{qn}", tag="psB")
            for qs in range(QC // P):
                nc.tensor.transpose(
                    pst[:, qs * VW2:qs * VW2 + VW],
                    ct[:, qs * P:(qs + 1) * P],
                    id_bf[0:VW, 0:VW],
                )
            rec = smallp.tile([P, 4], f32, name=f"r{h}_{qn}", tag="rec")
            pst4 = pst.rearrange("p (c w) -> p c w", w=VW2)
            nc.vector.reciprocal(rec, pst4[:, 0:4, DH])
            for qs in range(QC // P):
                qi = qn * (QC // P) + qs
                if split_muls and qs % 2:
                    nc.scalar.mul(
                        ctx_sb[qi][:, h * DH:(h + 1) * DH],
                        pst[:, qs * VW2:qs * VW2 + DH],
                        rec[:, qs:qs + 1],
                    )
                else:
                    nc.vector.tensor_scalar_mul(
                        ctx_sb[qi][:, h * DH:(h + 1) * DH],
                        pst[:, qs * VW2:qs * VW2 + DH],
                        rec[:, qs:qs + 1],
                    )

        def emit_out_cols(c0, c1, st0=0, st1=NS, eng=None):
            # columns [c0, c1) final for rows [st0*P, st1*P): one 3D DMA.
            # Rides the Pool SWDGE queue by default so output transfers
            # never queue behind the W loads on the SP ring's queue.
            (eng or nc.gpsimd).dma_start(
                out=o_d[st0 * P:st1 * P, c0:c1].rearrange(
                    "(st p) c -> p st c", p=P
                ),
                in_=ctx_all[:, st0:st1, c0:c1],
            )

        # ---- startup: X^T, QK0 (interleaved with the last four X^T
        # tiles: the kt n=0 half only needs X^T columns 0:512)
        qt0 = qktp.tile([P, S], bf16, name="qt0", tag="qt")
        kt0 = qktp.tile([P, S], bf16, name="kt0", tag="kt")
        ps_k = psA.tile([P, S], f32, name="pq0_1", tag="psA")
        ps_q = psA.tile([P, S], f32, name="pq0_0", tag="psA")
        for k in range(NK):
            nc.tensor.matmul(
                ps_k[:, 0:QC], wbf0[1][:, k, :], xt[:, k, 0:QC],
                start=(k == 0), stop=(k == NK - 1),
            )
        emit_xt(4)
        emit_xt(5)
        for k in range(NK):
            nc.tensor.matmul(
                ps_q[:, 0:QC], wbf0[0][:, k, :], xt[:, k, 0:QC],
                start=(k == 0), stop=(k == NK - 1),
            )
        emit_xt(6)
        emit_xt(7)
        for k in range(NK):
            nc.tensor.matmul(
                ps_k[:, QC:S], wbf0[1][:, k, :], xt[:, k, QC:S],
                start=(k == 0), stop=(k == NK - 1),
            )
        nc.vector.tensor_copy(kt0, ps_k)
        for k in range(NK):
            nc.tensor.matmul(
                ps_q[:, QC:S], wbf0[0][:, k, :], xt[:, k, QC:S],
                start=(k == 0), stop=(k == NK - 1),
            )
        nc.vector.tensor_copy(qt0, ps_q)

        # pair-1 W casts first (data already landed), then W_v casts
        wbf1 = []
        for t, wf in enumerate(wfs1):
            wb = wqkp.tile([P, NK, P], bf16, name=f"wb1_{t}", tag="wb")
            nc.vector.tensor_copy(wb, wf)
            wbf1.append(wb)
        for q in range(4):
            nc.vector.tensor_copy(wvb[:, 2 * q:2 * q + 2, :], wvfs[q])

        # ---- pair 0: interleaved scores(0,1) with QK1 chunks + V' fillers
        # (chunk pairs c0+c1 emitted whole so the psB ring alternates
        # cleanly with the V' tiles — no cross-tile open-group interleave)
        qt1, kt1, chunks1 = qk_chunks(1, wbf1)
        wf2 = qk_load_dma(2)
        es0, es1 = scores_tiles(0), scores_tiles(1)
        for k in range(NK):
            emit_scores_unit(0, k, es0, es1, qt0, kt0)
            if k % 2 == 1:
                chunks1[k - 1]()
                chunks1[k]()
            emit_v_st(k)
        wbf_store = {2: qk_cast(2, wf2)}

        # ---- steady-state pairs 1..7. es sets are a 3-deep ring: head 2hp
        # reuses head 2hp-3's tiles (PV'd mid pair hp-1) and head 2hp+1
        # reuses 2hp-2's, whose PV must therefore run BEFORE this pair's
        # units (the between-block).
        es_sets = {0: es0, 1: es1}
        ct_store = {}
        wf_store = {}
        qt_cur, kt_cur = qt1, kt1
        for hp in range(1, NHP):
            h_even_prev = 2 * hp - 2      # even head of pair hp-1
            h_odd_prev = 2 * hp - 1       # odd head of pair hp-1
            h_odd_prev2 = 2 * hp - 3      # odd head of pair hp-2
            qt_t, kt_t = qt_cur, kt_cur

            # between-block: PV of the even head of the previous pair
            es_even = es_sets.pop(h_even_prev)
            cts_prev = [emit_pv_half(h_even_prev, es_even, 0),
                        emit_pv_half(h_even_prev, es_even, 1)]

            if hp + 1 < NHP:
                qtn, ktn, chunks = qk_chunks(hp + 1, wbf_store.pop(hp + 1))
            else:
                qtn = ktn = None
                chunks = [None] * 8
            if hp + 2 < NHP:
                wf_store[hp + 2] = qk_load_dma(hp + 2)

            es_odd = es_sets.pop(h_odd_prev)
            es_a = scores_tiles(2 * hp)
            es_b = scores_tiles(2 * hp + 1)

            def unit(k, *fillers):
                emit_scores_unit(hp, k, es_a, es_b, qt_t, kt_t)
                for f in fillers:
                    if f is not None:
                        f()

            if hp >= 2:
                ct_p2 = ct_store.pop(h_odd_prev2)
            if hp < NHP - 1:
                unit(0, chunks[0],
                     None if hp < 2 else lambda: emit_ctxt_qn(
                         h_odd_prev2, ct_p2[0], 0))
                unit(1, chunks[1],
                     None if hp < 2 else lambda: emit_ctxt_qn(
                         h_odd_prev2, ct_p2[1], 1))
                unit(2, lambda: ct_store.setdefault(h_odd_prev, []).append(
                    emit_pv_half(h_odd_prev, es_odd, 0)), chunks[2])
                unit(3, chunks[3],
                     lambda: emit_ctxt_qn(h_even_prev, cts_prev[0], 0))
                unit(4, chunks[4],
                     lambda: emit_ctxt_qn(h_even_prev, cts_prev[1], 1))
                unit(5, lambda: ct_store[h_odd_prev].append(
                    emit_pv_half(h_odd_prev, es_odd, 1)), chunks[5])
                unit(6, chunks[6])
                unit(7, chunks[7])
            else:
                # last pair: no QK chunks
                unit(0, lambda: emit_ctxt_qn(h_odd_prev2, ct_p2[0], 0))
                unit(1, lambda: emit_ctxt_qn(h_odd_prev2, ct_p2[1], 1))
                unit(2, lambda: ct_store.setdefault(h_odd_prev, []).append(
                    emit_pv_half(h_odd_prev, es_odd, 0)))
                unit(3, lambda: emit_ctxt_qn(h_even_prev, cts_prev[0], 0))
                unit(4, lambda: emit_ctxt_qn(h_even_prev, cts_prev[1], 1))
                unit(5, lambda: ct_store[h_odd_prev].append(
                    emit_pv_half(h_odd_prev, es_odd, 1)))
                unit(6, None)
                unit(7, None)
            if hp + 2 < NHP:
                wbf_store[hp + 2] = qk_cast(hp + 2, wf_store.pop(hp + 2))
            es_sets[2 * hp] = es_a
            es_sets[2 * hp + 1] = es_b
            qt_cur, kt_cur = qtn, ktn
            if hp == 3:
                emit_out_cols(0, 5 * DH)
            elif hp == 5:
                emit_out_cols(5 * DH, 9 * DH)
            elif hp == 7:
                emit_out_cols(9 * DH, 13 * DH)

        # ---- tail: heads 13 (ctxT), 14, 15 (PV + ctxT). The last columns
        # go out as two row-half DMAs on different rings (parallel queues),
        # the first fired as soon as the qn0 muls land.
        es14 = es_sets.pop(14)
        es15 = es_sets.pop(15)
        ct13 = ct_store.pop(13)
        ct14_0 = emit_pv_half(14, es14, 0)
        emit_ctxt_qn(13, ct13[0], 0)
        ct14_1 = emit_pv_half(14, es14, 1)
        emit_ctxt_qn(14, ct14_0, 0, split_muls=True)
        ct15_0 = emit_pv_half(15, es15, 0)
        emit_ctxt_qn(13, ct13[1], 1)
        ct15_1 = emit_pv_half(15, es15, 1)
        emit_ctxt_qn(15, ct15_0, 0, split_muls=True)
        emit_out_cols(13 * DH, 16 * DH, 0, NS // 2, eng=nc.gpsimd)
        emit_ctxt_qn(14, ct14_1, 1, split_muls=True)
        emit_ctxt_qn(15, ct15_1, 1, split_muls=True)
        emit_out_cols(13 * DH, 16 * DH, NS // 2, NS, eng=nc.scalar)

    nc.finalize()
    return nc


def _get_nc():
    if "nc" not in _NC_CACHE:
        _NC_CACHE["nc"] = _build_nc()
    return _NC_CACHE["nc"]


def _run(hidden_states, attention_mask, qkv_weight, trace=False, **trace_kw):
    from concourse.bass_utils import run_bass_kernel_spmd

    nc = _get_nc()
    hidden = np.ascontiguousarray(np.asarray(hidden_states, dtype=np.float32))
    mask = np.ascontiguousarray(
        np.asarray(attention_mask, dtype=np.float32).reshape(B, S)
    )
    w = np.ascontiguousarray(np.asarray(qkv_weight, dtype=np.float32))
    in_maps = [
        {"x": hidden[b], "w": w, "m": mask[b]} for b in range(B)
    ]
    res = run_bass_kernel_spmd(nc, in_maps, list(range(B)), trace=trace, **trace_kw)
    out = np.stack([np.asarray(res.results[b]["o"]) for b in range(B)], axis=0)
    return out.astype(np.float32), res


def kernel(hidden_states, attention_mask, qkv_weight):
    out, _ = _run(hidden_states, attention_mask, qkv_weight, trace=False)
    return out


if __name__ == "__main__":
    _build_nc()
    print("build ok")


# revision 43
# speedup vs baseline: 1.1083x; 1.0114x over previous
"""BERT self-attention (B=8, S=1024, D=1024, H=16, Dh=64) on 8 NeuronCores.

Sharding: pure data parallel — core b handles batch element b (B == n_cores),
qkv_weight replicated. No collectives.

Per-core dataflow (all matmuls bf16 with fp32 PSUM accumulation):
  1. X [S,D] loaded first (prefetched 4 deep), cast to bf16 (DVE),
     PE-transposed into X^T [D,S] in groups of 4 chunks per PSUM unload;
     unloads alternate between DVE and ACT (idle early).
  2. W_v loaded+cast up front as [128, kt, 1024]; V computed into 2-bank
     [128,1024] PSUM tiles with stationary X^T chunks (128 matmuls), laid
     out as V' [S, H*(Dh+1)] where each head's 65th column carries
     exp(mask): softmax(s + m) == exp(s)*exp(m) normalized, so the additive
     mask is an exact per-key row scaling of V', and the extra column makes
     the PV matmul emit softmax denominators for free.
  3. Per head pair: W_q/W_k column slices loaded one pair ahead, Q^T,K^T
     computed as [features, S] into 2-bank PSUM tiles (one DVE unload each).
  4. Scores run PAIR-INTERLEAVED: head a (Q^T/K^T rows 0:64, PE row group
     h0) and head b (rows 64:128, row group h64) alternate matmuls per
     k-chunk, so the two 64-contraction matmuls execute CONCURRENTLY in
     disjoint halves of the 128x128 PE array (~1.9x on the scores stage).
     ACT computes exp(0.125*s) PSUM->SBUF(bf16) per [128,1024] tile.
  5. ctx'^T [65,S_q] = V'.T @ expS^T per head; copied to SBUF bf16 (DVE),
     PE-transposed (bf16) back to [S_q,65] four chunks per PSUM tile, one
     strided reciprocal per 4 denominators, cols 0..63 scaled by 1/col64
     on DVE, keeping ACT exp-only.
  6. ctx assembled [S, D] fp32, DMA'd out in column groups as head groups
     complete via the Pool-engine SWDGE queue (parallel to the input loads
     on the SP/ACT HWDGE queues); the final columns leave as two row-half
     DMAs on different queues, the first fired as soon as the qn0
     normalizations land.

Scores concurrency detail: each scores psA tile holds one qn half of BOTH
heads ([a | b]) and ONE 1024-wide exp drains it into a [p, head, s] pair
tile, so all four matmuls of a unit become schedule-ready at the same
instant — the Tile scheduler then places the h0/h64 matmuls back-to-back
(measured 4 ns apart on HW) — and ACT pays half the per-instruction
PSUM-access overhead of split 512-wide exps.

Schedule: exp-score pair tiles are a 2-deep ring of pair-sets. Per pair hp, 8 scores
units (a,b interleaved per k) with fillers woven between: PV(2hp-2) both
halves ride unit 0 (they free the es set that head 2hp+1 overwrites, and
keep ACT saturated instead of idling through a separate pre-pair block —
the unit-0 b-head exps wait only on PV(2hp-2) qn1's first k-read, which
the scheduler orders ahead of them), then PV(2hp-1) halves, QK chunks for
pair hp+1, ctxT of heads 2hp-3 / 2hp-2, and the pair hp+2 weight loads
(DMA at pair start, DVE cast at pair end to keep the strict-FIFO DVE
queue from stalling on the DMA semaphore). Input DMA is spread over both
HWDGE queues: X tiles 0-5 then W_v on the SP ring; masks, pair-0 W, then
(emitted after the pair-0 cast so their staging-slot wait cannot block
the ACT stream) pair-1 W and X tiles 6-7 on the ACT ring.

No max-subtraction in softmax: scores*scale is bounded (|x| <~ 4 for this
problem's scale) and exp runs in fp32 on ACT.
"""

import sys

import numpy as np

_REPO = "/opt/trn_rl_repo"
if _REPO not in sys.path:
    sys.path.insert(0, _REPO)

B, S, D, H, DH = 8, 1024, 1024, 16, 64
P = 128
NS = S // P          # seq tiles
NK = D // P          # contraction tiles
NHP = H // 2         # head pairs
NQ = 2               # 512-wide S_q chunks
QC = S // NQ         # 512
SCALE = 1.0 / 8.0    # 1/sqrt(DH)
VW = DH + 1          # V' live width per head (extra denominator column)
VP = DH + 2          # V' stored stride per head (pad for 4B-aligned slices)

_NC_CACHE = {}


def _build_nc():
    import concourse.bass as bass
    import concourse.tile as tile
    from concourse import bacc, mybir
    from concourse.masks import make_identity
    from contextlib import ExitStack

    f32 = mybir.dt.float32
    bf16 = mybir.dt.bfloat16
    Exp = mybir.ActivationFunctionType.Exp

    nc = bacc.Bacc("TRN2", target_bir_lowering=False, debug=False)
    x_d = nc.declare_dram_parameter("x", [S, D], f32, isOutput=False)
    w_d = nc.declare_dram_parameter("w", [D, 3 * D], f32, isOutput=False)
    m_d = nc.declare_dram_parameter("m", [S], f32, isOutput=False)
    o_d = nc.declare_dram_parameter("o", [S, D], f32, isOutput=True)

    with tile.TileContext(nc) as tc, ExitStack() as es:
        const = es.enter_context(tc.tile_pool(name="const", bufs=1))
        maskp = es.enter_context(tc.tile_pool(name="maskp", bufs=NS))
        xtp = es.enter_context(tc.tile_pool(name="xtp", bufs=1))
        vp = es.enter_context(tc.tile_pool(name="vp", bufs=NS))
        ctxp = es.enter_context(tc.tile_pool(name="ctxp", bufs=1))
        xstage = es.enter_context(tc.tile_pool(name="xstage", bufs=2))
        wvstage = es.enter_context(tc.tile_pool(name="wvstage", bufs=2))
        wvp = es.enter_context(tc.tile_pool(name="wvp", bufs=1))
        wstage = es.enter_context(tc.tile_pool(name="wstage", bufs=4))
        wqkp = es.enter_context(tc.tile_pool(name="wqkp", bufs=4))
        qktp = es.enter_context(tc.tile_pool(name="qktp", bufs=2))
        esp = es.enter_context(tc.tile_pool(name="esp", bufs=3 * NK))
        ctp = es.enter_context(tc.tile_pool(name="ctp", bufs=4))
        smallp = es.enter_context(tc.tile_pool(name="smallp", bufs=8))
        psA = es.enter_context(tc.tile_pool(name="psA", bufs=2, space="PSUM"))
        psB = es.enter_context(tc.tile_pool(name="psB", bufs=2, space="PSUM"))
        psC = es.enter_context(tc.tile_pool(name="psC", bufs=2, space="PSUM"))

        id_bf = const.tile([P, P], bf16, name="id_bf")
        make_identity(nc, id_bf)
        ones16 = const.tile([P, H], bf16, name="ones16")
        nc.vector.memset(ones16, 1.0)

        # persistent tensors
        xt = xtp.tile([P, NK, S], bf16, name="xt")  # X^T: [d-part, kt, s]
        v_sb = [vp.tile([P, H * VP], bf16, name=f"v{st}", tag="v") for st in range(NS)]
        ctx_all = ctxp.tile([P, NS, D], f32, name="ctx_all")
        ctx_sb = [ctx_all[:, st, :] for st in range(NS)]

        # X loads first on the SP ring (startup critical path); pair-0 W
        # slices after the first four tiles so QK0 can interleave with X^T
        xfs = []
        for i in range(NS):
            xf = xstage.tile([P, D], f32, name=f"xf{i}", tag="xf", bufs=4)
            nc.sync.dma_start(
                out=xf[:, 0:QC], in_=x_d[i * P:(i + 1) * P, 0:QC]
            )
            nc.sync.dma_start(
                out=xf[:, QC:D], in_=x_d[i * P:(i + 1) * P, QC:D]
            )
            xfs.append(xf)

        # All ACT-ring DMA triggers fire BEFORE any ACT compute is queued
        # (strict FIFO): masks, pair-0/1 W slices, then W_v — a DMA queue
        # parallel to the X load on the SP ring.
        msks = []
        for st in range(NS):
            msk = maskp.tile([P, 1], f32, name=f"msk{st}", tag="msk")
            nc.scalar.dma_start(
                out=msk,
                in_=m_d[st * P:(st + 1) * P].rearrange("(p o) -> p o", o=1),
            )
            msks.append(msk)

        # X cast + PE transpose; PSUM unloads in groups of 4 chunks,
        # alternating DVE / ACT
        def emit_xt(i):
            xb = xstage.tile([P, D], bf16, name=f"xb{i}", tag="xb", bufs=1)
            for g in range(2):
                nc.vector.tensor_copy(
                    xb[:, g * QC:(g + 1) * QC], xfs[i][:, g * QC:(g + 1) * QC]
                )
                pst = psB.tile([P, 4 * P], bf16, name=f"px{i}_{g}", tag="psB")
                for c in range(4):
                    j = 4 * g + c
                    nc.tensor.transpose(
                        pst[:, c * P:(c + 1) * P], xb[:, j * P:(j + 1) * P], id_bf
                    )
                eng = nc.vector if (2 * i + g) % 2 else nc.scalar
                dst = xt[:, 4 * g:4 * g + 4, i * P:(i + 1) * P]
                srcp = pst.rearrange("p (c q) -> p c q", c=4)
                if eng is nc.vector:
                    eng.tensor_copy(dst, srcp)
                else:
                    eng.copy(dst, srcp)

        def qk_load_dma(hp, eng=None):
            # W_q/W_k column slice DMAs for this head pair (SP ring by
            # default; startup loads ride the ACT ring so they do not queue
            # behind the 4MB X load on the SP ring's DMA queue)
            wfs = []
            for t, base in enumerate((hp * P, D + hp * P)):
                wf = wstage.tile([P, NK, P], f32, name=f"wf{hp}_{t}", tag="wf")
                (eng or nc.sync).dma_start(
                    out=wf,
                    in_=w_d[:, base:base + P].rearrange("(kt p) c -> p kt c", p=P),
                )
                wfs.append(wf)
            return wfs

        def qk_cast(hp, wfs):
            # bf16 casts (DVE) — issued well after the DMA so the strict-FIFO
            # DVE queue never stalls on the DMA semaphore
            wbf = []
            for t, wf in enumerate(wfs):
                wb = wqkp.tile([P, NK, P], bf16, name=f"wb{hp}_{t}", tag="wb")
                nc.vector.tensor_copy(wb, wf)
                wbf.append(wb)
            return wbf

        def qk_load(hp):
            return qk_cast(hp, qk_load_dma(hp))

        # pair-0/1 W DMAs on the ACT ring (queued right behind the tiny
        # mask DMAs: W0 lands ~t5us, long before QK0 needs it)
        wfs0 = qk_load_dma(0, eng=nc.scalar)

        # W_v full load on the SP ring behind X (lands ~t26us, before the
        # pair-0 V' matmuls need it)
        wvb = wvp.tile([P, NK, D], bf16, name="wvb")
        wvfs = []
        for q in range(4):
            wvf = wvstage.tile([P, 2, D], f32, name=f"wvf{q}", tag="wvf")
            nc.sync.dma_start(
                out=wvf,
                in_=w_d[2 * q * P:(2 * q + 2) * P, 2 * D:3 * D].rearrange(
                    "(kt p) c -> p kt c", p=P
                ),
            )
            wvfs.append(wvf)

        # exp(mask) per seq tile — the first ACT compute in the queue
        em = []
        for st in range(NS):
            emt = maskp.tile([P, 1], f32, name=f"em{st}", tag="em")
            nc.scalar.activation(emt, msks[st], Exp)
            em.append(emt)

        # X^T for the first four tiles, then the pair-0 W cast
        for i in range(4):
            emit_xt(i)
        wbf0 = qk_cast(0, wfs0)
        # pair-1 W triggers AFTER the pair-0 cast: their wstage-slot wait
        # (ring of 3, slot freed by that cast) must not block the ACT
        # stream's em/vcols/X^T-unload ops behind it
        wfs1 = qk_load_dma(1, eng=nc.scalar)

        # V' denominator columns = exp(mask) per key row (Pool engine)
        for st in range(NS):
            vcols = v_sb[st].rearrange("p (h c) -> p h c", h=H)[:, :, DH]
            nc.scalar.mul(vcols, ones16, em[st])

        def qk_chunks(hp, wbf):
            # QK as 8 four-matmul chunks into 1-bank psB tiles; each
            # (wsel, n) group is two chunks + a DVE unload, interleavable
            # between score units
            qt_t = qktp.tile([P, S], bf16, name=f"qt{hp}", tag="qt")
            kt_t = qktp.tile([P, S], bf16, name=f"kt{hp}", tag="kt")
            chunks = []
            for wsel, dest in ((1, kt_t), (0, qt_t)):
                for n in range(NQ):
                    cell = {}

                    def c0(cell=cell, wsel=wsel, n=n):
                        ps = psB.tile(
                            [P, QC], f32, name=f"pq{hp}_{wsel}_{n}", tag="psB"
                        )
                        cell["ps"] = ps
                        for k in range(4):
                            nc.tensor.matmul(
                                ps,
                                wbf[wsel][:, k, :],
                                xt[:, k, n * QC:(n + 1) * QC],
                                start=(k == 0),
                                stop=False,
                            )

                    def c1(cell=cell, wsel=wsel, n=n, dest=dest):
                        ps = cell["ps"]
                        for k in range(4, NK):
                            nc.tensor.matmul(
                                ps,
                                wbf[wsel][:, k, :],
                                xt[:, k, n * QC:(n + 1) * QC],
                                start=False,
                                stop=(k == NK - 1),
                            )
                        nc.vector.tensor_copy(
                            dest[:, n * QC:(n + 1) * QC], ps
                        )

                    chunks.append(c0)
                    chunks.append(c1)
            return qt_t, kt_t, chunks

        def emit_v_st(st):
            # V' [S, H*(Dh+2) padded]: stationary X^T chunks, 512-wide W_v;
            # per-key exp(mask) row scaling on the Pool engine
            for half in range(2):
                ps = psB.tile([P, QC], f32, name=f"pv{st}_{half}", tag="psB")
                for k in range(NK):
                    nc.tensor.matmul(
                        ps,
                        xt[:, k, st * P:(st + 1) * P],
                        wvb[:, k, half * QC:(half + 1) * QC],
                        start=(k == 0),
                        stop=(k == NK - 1),
                    )
                vdst = v_sb[st].rearrange("p (h c) -> p h c", h=H)[
                    :, half * 8:(half + 1) * 8, 0:DH
                ]
                vsrc = ps.rearrange("p (h c) -> p h c", h=8)
                nc.scalar.mul(vdst, vsrc, em[st])

        def scores_tiles(h):
            return [
                esp.tile([P, S], bf16, name=f"e{h}_{k}", tag="es") for k in range(NK)
            ]

        def emit_scores_unit(hp, k, esa, esb_, qt_t, kt_t):
            # one k-chunk of BOTH heads of the pair. Each psA tile holds one
            # qn half of BOTH heads ([a | b]), so all four matmuls of a unit
            # become schedule-ready together (the previous unit's exps free
            # both halves at once) and the a/b matmuls run CONCURRENTLY in
            # disjoint PE row groups (a: Q^T/K^T rows 0:64 / row group h0,
            # b: rows 64:128 / h64).
            for qn in range(NQ):
                ps = psA.tile([P, S], f32, name=f"s{hp}_{k}_{qn}", tag="psA")
                nc.tensor.matmul(
                    ps[:, 0:QC],
                    kt_t[0:DH, k * P:(k + 1) * P],
                    qt_t[0:DH, qn * QC:(qn + 1) * QC],
                    start=True,
                    stop=True,
                )
                nc.tensor.matmul(
                    ps[:, QC:S],
                    kt_t[DH:P, k * P:(k + 1) * P],
                    qt_t[DH:P, qn * QC:(qn + 1) * QC],
                    start=True,
                    stop=True,
                )
                nc.scalar.activation(
                    esa[k][:, qn * QC:(qn + 1) * QC], ps[:, 0:QC],
                    Exp, scale=SCALE,
                )
                nc.scalar.activation(
                    esb_[k][:, qn * QC:(qn + 1) * QC], ps[:, QC:S],
                    Exp, scale=SCALE,
                )

        def emit_pv_half(h, esb_, qn):
            # ctx'^T [65, S_q] = V'.T @ expS^T; SBUF bf16 copy (DVE)
            psc = psC.tile([VW, QC], f32, name=f"c{h}_{qn}", tag="psC")
            for k in range(NK):
                nc.tensor.matmul(
                    psc,
                    v_sb[k][:, h * VP:h * VP + VW],
                    esb_[k][:, qn * QC:(qn + 1) * QC],
                    start=(k == 0),
                    stop=(k == NK - 1),
                )
            ct = ctp.tile([VW, QC], bf16, name=f"ct{h}_{qn}", tag="ct")
            nc.vector.tensor_copy(ct, psc)
            return ct

        def emit_ctxt_qn(h, ct, qn, split_muls=False):
            # 4 bf16 PE transposes per PSUM tile back to [S_q, 65];
            # one strided reciprocal per 4 denominators; normalize on DVE
            VW2 = VW + 1  # 66: keeps each chunk's PSUM byte offset 4B-aligned
            pst = psB.tile([P, 4 * VW2], bf16, name=f"pt{h}_{qn}", tag="psB")
            for qs in range(QC // P):
                nc.tensor.transpose(
                    pst[:, qs * VW2:qs * VW2 + VW],
                    ct[:, qs * P:(qs + 1) * P],
                    id_bf[0:VW, 0:VW],
                )
            rec = smallp.tile([P, 4], f32, name=f"r{h}_{qn}", tag="rec")
            pst4 = pst.rearrange("p (c w) -> p c w", w=VW2)
            nc.vector.reciprocal(rec, pst4[:, 0:4, DH])
            for qs in range(QC // P):
                qi = qn * (QC // P) + qs
                if split_muls and qs % 2:
                    nc.scalar.mul(
                        ctx_sb[qi][:, h * DH:(h + 1) * DH],
                        pst[:, qs * VW2:qs * VW2 + DH],
                        rec[:, qs:qs + 1],
                    )
                else:
                    nc.vector.tensor_scalar_mul(
                        ctx_sb[qi][:, h * DH:(h + 1) * DH],
                        pst[:, qs * VW2:qs * VW2 + DH],
                        rec[:, qs:qs + 1],
                    )

        def emit_out_cols(c0, c1, st0=0, st1=NS, eng=None):
            # columns [c0, c1) final for rows [st0*P, st1*P): one 3D DMA.
            # Rides the Pool SWDGE queue by default so output transfers
            # never queue behind the W loads on the SP ring's queue.
            (eng or nc.gpsimd).dma_start(
                out=o_d[st0 * P:st1 * P, c0:c1].rearrange(
                    "(st p) c -> p st c", p=P
                ),
                in_=ctx_all[:, st0:st1, c0:c1],
            )

        # ---- startup: X^T, QK0 (interleaved with the last four X^T
        # tiles: the kt n=0 half only needs X^T columns 0:512)
        qt0 = qktp.tile([P, S], bf16, name="qt0", tag="qt")
        kt0 = qktp.tile([P, S], bf16, name="kt0", tag="kt")
        ps_k = psA.tile([P, S], f32, name="pq0_1", tag="psA")
        ps_q = psA.tile([P, S], f32, name="pq0_0", tag="psA")
        for k in range(NK):
            nc.tensor.matmul(
                ps_k[:, 0:QC], wbf0[1][:, k, :], xt[:, k, 0:QC],
                start=(k == 0), stop=(k == NK - 1),
            )
        emit_xt(4)
        emit_xt(5)
        for k in range(NK):
            nc.tensor.matmul(
                ps_q[:, 0:QC], wbf0[0][:, k, :], xt[:, k, 0:QC],
                start=(k == 0), stop=(k == NK - 1),
            )
        emit_xt(6)
        emit_xt(7)
        for k in range(NK):
            nc.tensor.matmul(
                ps_k[:, QC:S], wbf0[1][:, k, :], xt[:, k, QC:S],
                start=(k == 0), stop=(k == NK - 1),
            )
        nc.vector.tensor_copy(kt0, ps_k)
        for k in range(NK):
            nc.tensor.matmul(
                ps_q[:, QC:S], wbf0[0][:, k, :], xt[:, k, QC:S],
                start=(k == 0), stop=(k == NK - 1),
            )
        nc.vector.tensor_copy(qt0, ps_q)

        # pair-1 W casts first (data already landed), then W_v casts
        wbf1 = []
        for t, wf in enumerate(wfs1):
            wb = wqkp.tile([P, NK, P], bf16, name=f"wb1_{t}", tag="wb")
            nc.vector.tensor_copy(wb, wf)
            wbf1.append(wb)
        for q in range(4):
            nc.vector.tensor_copy(wvb[:, 2 * q:2 * q + 2, :], wvfs[q])

        # ---- pair 0: interleaved scores(0,1) with QK1 chunks + V' fillers
        # (chunk pairs c0+c1 emitted whole so the psB ring alternates
        # cleanly with the V' tiles — no cross-tile open-group interleave)
        qt1, kt1, chunks1 = qk_chunks(1, wbf1)
        wf2 = qk_load_dma(2)
        es0, es1 = scores_tiles(0), scores_tiles(1)
        for k in range(NK):
            emit_scores_unit(0, k, es0, es1, qt0, kt0)
            if k % 2 == 1:
                chunks1[k - 1]()
                chunks1[k]()
            emit_v_st(k)
        wbf_store = {2: qk_cast(2, wf2)}

        # ---- steady-state pairs 1..7. es sets are a 3-deep ring: head 2hp
        # reuses head 2hp-3's tiles (PV'd mid pair hp-1) and head 2hp+1
        # reuses 2hp-2's, whose PV must therefore run BEFORE this pair's
        # units (the between-block).
        es_sets = {0: es0, 1: es1}
        ct_store = {}
        wf_store = {}
        qt_cur, kt_cur = qt1, kt1
        for hp in range(1, NHP):
            h_even_prev = 2 * hp - 2      # even head of pair hp-1
            h_odd_prev = 2 * hp - 1       # odd head of pair hp-1
            h_odd_prev2 = 2 * hp - 3      # odd head of pair hp-2
            qt_t, kt_t = qt_cur, kt_cur

            # between-block: PV of the even head of the previous pair
            es_even = es_sets.pop(h_even_prev)
            cts_prev = [emit_pv_half(h_even_prev, es_even, 0),
                        emit_pv_half(h_even_prev, es_even, 1)]

            if hp + 1 < NHP:
                qtn, ktn, chunks = qk_chunks(hp + 1, wbf_store.pop(hp + 1))
            else:
                qtn = ktn = None
                chunks = [None] * 8
            if hp + 2 < NHP:
                wf_store[hp + 2] = qk_load_dma(hp + 2)

            es_odd = es_sets.pop(h_odd_prev)
            es_a = scores_tiles(2 * hp)
            es_b = scores_tiles(2 * hp + 1)

            def unit(k, *fillers):
                emit_scores_unit(hp, k, es_a, es_b, qt_t, kt_t)
                for f in fillers:
                    if f is not None:
                        f()

            if hp >= 2:
                ct_p2 = ct_store.pop(h_odd_prev2)
            if hp < NHP - 1:
                unit(0, chunks[0],
                     None if hp < 2 else lambda: emit_ctxt_qn(
                         h_odd_prev2, ct_p2[0], 0))
                unit(1, chunks[1],
                     None if hp < 2 else lambda: emit_ctxt_qn(
                         h_odd_prev2, ct_p2[1], 1))
                unit(2, lambda: ct_store.setdefault(h_odd_prev, []).append(
                    emit_pv_half(h_odd_prev, es_odd, 0)), chunks[2])
                unit(3, chunks[3],
                     lambda: emit_ctxt_qn(h_even_prev, cts_prev[0], 0))
                unit(4, chunks[4],
                     lambda: emit_ctxt_qn(h_even_prev, cts_prev[1], 1))
                unit(5, lambda: ct_store[h_odd_prev].append(
                    emit_pv_half(h_odd_prev, es_odd, 1)), chunks[5])
                unit(6, chunks[6])
                unit(7, chunks[7])
            else:
                # last pair: no QK chunks
                unit(0, lambda: emit_ctxt_qn(h_odd_prev2, ct_p2[0], 0))
                unit(1, lambda: emit_ctxt_qn(h_odd_prev2, ct_p2[1], 1))
                unit(2, lambda: ct_store.setdefault(h_odd_prev, []).append(
                    emit_pv_half(h_odd_prev, es_odd, 0)))
                unit(3, lambda: emit_ctxt_qn(h_even_prev, cts_prev[0], 0))
                unit(4, lambda: emit_ctxt_qn(h_even_prev, cts_prev[1], 1))
                unit(5, lambda: ct_store[h_odd_prev].append(
                    emit_pv_half(h_odd_prev, es_odd, 1)))
                unit(6, None)
                unit(7, None)
            if hp + 2 < NHP:
                wbf_store[hp + 2] = qk_cast(hp + 2, wf_store.pop(hp + 2))
            es_sets[2 * hp] = es_a
            es_sets[2 * hp + 1] = es_b
            qt_cur, kt_cur = qtn, ktn
            if hp == 3:
                emit_out_cols(0, 5 * DH)
            elif hp == 5:
                emit_out_cols(5 * DH, 9 * DH)
            elif hp == 7:
                emit_out_cols(9 * DH, 13 * DH)

        # ---- tail: heads 13 (ctxT), 14, 15 (PV + ctxT). The last columns
        # go out as two row-half DMAs on different rings (parallel queues),
        # the first fired as soon as the qn0 muls land.
        es14 = es_sets.pop(14)
        es15 = es_sets.pop(15)
        ct13 = ct_store.pop(13)
        ct14_0 = emit_pv_half(14, es14, 0)
        emit_ctxt_qn(13, ct13[0], 0)
        ct14_1 = emit_pv_half(14, es14, 1)
        emit_ctxt_qn(14, ct14_0, 0, split_muls=True)
        ct15_0 = emit_pv_half(15, es15, 0)
        emit_ctxt_qn(13, ct13[1], 1)
        ct15_1 = emit_pv_half(15, es15, 1)
        emit_ctxt_qn(15, ct15_0, 0, split_muls=True)
        emit_out_cols(13 * DH, 16 * DH, 0, NS // 2, eng=nc.gpsimd)
        emit_ctxt_qn(14, ct14_1, 1, split_muls=True)
        emit_ctxt_qn(15, ct15_1, 1, split_muls=True)
        emit_out_cols(13 * DH, 16 * DH, NS // 2, NS, eng=nc.scalar)

    nc.finalize()
    return nc


def _get_nc():
    if "nc" not in _NC_CACHE:
        _NC_CACHE["nc"] = _build_nc()
    return _NC_CACHE["nc"]


def _run(hidden_states, attention_mask, qkv_weight, trace=False, **trace_kw):
    from concourse.bass_utils import run_bass_kernel_spmd

    nc = _get_nc()
    hidden = np.ascontiguousarray(np.asarray(hidden_states, dtype=np.float32))
    mask = np.ascontiguousarray(
        np.asarray(attention_mask, dtype=np.float32).reshape(B, S)
    )
    w = np.ascontiguousarray(np.asarray(qkv_weight, dtype=np.float32))
    in_maps = [
        {"x": hidden[b], "w": w, "m": mask[b]} for b in range(B)
    ]
    res = run_bass_kernel_spmd(nc, in_maps, list(range(B)), trace=trace, **trace_kw)
    out = np.stack([np.asarray(res.results[b]["o"]) for b in range(B)], axis=0)
    return out.astype(np.float32), res


def kernel(hidden_states, attention_mask, qkv_weight):
    out, _ = _run(hidden_states, attention_mask, qkv_weight, trace=False)
    return out


if __name__ == "__main__":
    _build_nc()
    print("build ok")


# revision 44
# speedup vs baseline: 1.1200x; 1.0106x over previous
"""BERT self-attention (B=8, S=1024, D=1024, H=16, Dh=64) on 8 NeuronCores.

Sharding: pure data parallel — core b handles batch element b (B == n_cores),
qkv_weight replicated. No collectives.

Per-core dataflow (all matmuls bf16 with fp32 PSUM accumulation):
  1. X [S,D] loaded first (prefetched 4 deep), cast to bf16 (DVE),
     PE-transposed into X^T [D,S] in groups of 4 chunks per PSUM unload;
     unloads alternate between DVE and ACT (idle early).
  2. W_v loaded+cast up front as [128, kt, 1024]; V computed into 2-bank
     [128,1024] PSUM tiles with stationary X^T chunks (128 matmuls), laid
     out as V' [S, H*(Dh+1)] where each head's 65th column carries
     exp(mask): softmax(s + m) == exp(s)*exp(m) normalized, so the additive
     mask is an exact per-key row scaling of V', and the extra column makes
     the PV matmul emit softmax denominators for free.
  3. Per head pair: W_q/W_k column slices loaded one pair ahead, Q^T,K^T
     computed as [features, S] into 2-bank PSUM tiles (one DVE unload each).
  4. Scores run PAIR-INTERLEAVED: head a (Q^T/K^T rows 0:64, PE row group
     h0) and head b (rows 64:128, row group h64) alternate matmuls per
     k-chunk, so the two 64-contraction matmuls execute CONCURRENTLY in
     disjoint halves of the 128x128 PE array (~1.9x on the scores stage).
     ACT computes exp(0.125*s) PSUM->SBUF(bf16) per [128,1024] tile.
  5. ctx'^T [65,S_q] = V'.T @ expS^T per head; copied to SBUF bf16 (DVE),
     PE-transposed (bf16) back to [S_q,65] four chunks per PSUM tile, one
     strided reciprocal per 4 denominators, cols 0..63 scaled by 1/col64
     on DVE, keeping ACT exp-only.
  6. ctx assembled [S, D] fp32, DMA'd out in column groups as head groups
     complete via the Pool-engine SWDGE queue (parallel to the input loads
     on the SP/ACT HWDGE queues); the final columns leave as two row-half
     DMAs on different queues, the first fired as soon as the qn0
     normalizations land.

Scores concurrency detail: each scores psA tile holds one qn half of BOTH
heads ([a | b]) and ONE 1024-wide exp drains it into a [p, head, s] pair
tile, so all four matmuls of a unit become schedule-ready at the same
instant — the Tile scheduler then places the h0/h64 matmuls back-to-back
(measured 4 ns apart on HW) — and ACT pays half the per-instruction
PSUM-access overhead of split 512-wide exps.

Schedule: exp-score pair tiles are a 2-deep ring of pair-sets. Per pair hp, 8 scores
units (a,b interleaved per k) with fillers woven between: PV(2hp-2) both
halves ride unit 0 (they free the es set that head 2hp+1 overwrites, and
keep ACT saturated instead of idling through a separate pre-pair block —
the unit-0 b-head exps wait only on PV(2hp-2) qn1's first k-read, which
the scheduler orders ahead of them), then PV(2hp-1) halves, QK chunks for
pair hp+1, ctxT of heads 2hp-3 / 2hp-2, and the pair hp+2 weight loads
(DMA at pair start, DVE cast at pair end to keep the strict-FIFO DVE
queue from stalling on the DMA semaphore). Input DMA is spread over both
HWDGE queues: X tiles 0-5 then W_v on the SP ring; masks, pair-0 W, then
(emitted after the pair-0 cast so their staging-slot wait cannot block
the ACT stream) pair-1 W and X tiles 6-7 on the ACT ring.

No max-subtraction in softmax: scores*scale is bounded (|x| <~ 4 for this
problem's scale) and exp runs in fp32 on ACT.
"""

import sys

import numpy as np

_REPO = "/opt/trn_rl_repo"
if _REPO not in sys.path:
    sys.path.insert(0, _REPO)

B, S, D, H, DH = 8, 1024, 1024, 16, 64
P = 128
NS = S // P          # seq tiles
NK = D // P          # contraction tiles
NHP = H // 2         # head pairs
NQ = 2               # 512-wide S_q chunks
QC = S // NQ         # 512
SCALE = 1.0 / 8.0    # 1/sqrt(DH)
VW = DH + 1          # V' live width per head (extra denominator column)
VP = DH + 2          # V' stored stride per head (pad for 4B-aligned slices)

_NC_CACHE = {}


def _build_nc():
    import concourse.bass as bass
    import concourse.tile as tile
    from concourse import bacc, mybir
    from concourse.masks import make_identity
    from contextlib import ExitStack

    f32 = mybir.dt.float32
    bf16 = mybir.dt.bfloat16
    Exp = mybir.ActivationFunctionType.Exp

    nc = bacc.Bacc("TRN2", target_bir_lowering=False, debug=False)
    x_d = nc.declare_dram_parameter("x", [S, D], f32, isOutput=False)
    w_d = nc.declare_dram_parameter("w", [D, 3 * D], f32, isOutput=False)
    m_d = nc.declare_dram_parameter("m", [S], f32, isOutput=False)
    o_d = nc.declare_dram_parameter("o", [S, D], f32, isOutput=True)

    with tile.TileContext(nc) as tc, ExitStack() as es:
        const = es.enter_context(tc.tile_pool(name="const", bufs=1))
        maskp = es.enter_context(tc.tile_pool(name="maskp", bufs=NS))
        xtp = es.enter_context(tc.tile_pool(name="xtp", bufs=1))
        vp = es.enter_context(tc.tile_pool(name="vp", bufs=NS))
        ctxp = es.enter_context(tc.tile_pool(name="ctxp", bufs=1))
        xstage = es.enter_context(tc.tile_pool(name="xstage", bufs=2))
        wvstage = es.enter_context(tc.tile_pool(name="wvstage", bufs=2))
        wvp = es.enter_context(tc.tile_pool(name="wvp", bufs=1))
        wstage = es.enter_context(tc.tile_pool(name="wstage", bufs=4))
        wqkp = es.enter_context(tc.tile_pool(name="wqkp", bufs=4))
        qktp = es.enter_context(tc.tile_pool(name="qktp", bufs=2))
        esp = es.enter_context(tc.tile_pool(name="esp", bufs=3 * NK))
        ctp = es.enter_context(tc.tile_pool(name="ctp", bufs=4))
        smallp = es.enter_context(tc.tile_pool(name="smallp", bufs=8))
        psA = es.enter_context(tc.tile_pool(name="psA", bufs=2, space="PSUM"))
        psB = es.enter_context(tc.tile_pool(name="psB", bufs=2, space="PSUM"))
        psC = es.enter_context(tc.tile_pool(name="psC", bufs=2, space="PSUM"))

        id_bf = const.tile([P, P], bf16, name="id_bf")
        make_identity(nc, id_bf)
        ones16 = const.tile([P, H], bf16, name="ones16")
        nc.vector.memset(ones16, 1.0)

        # persistent tensors
        xt = xtp.tile([P, NK, S], bf16, name="xt")  # X^T: [d-part, kt, s]
        v_sb = [vp.tile([P, H * VP], bf16, name=f"v{st}", tag="v") for st in range(NS)]
        ctx_all = ctxp.tile([P, NS, D], f32, name="ctx_all")
        ctx_sb = [ctx_all[:, st, :] for st in range(NS)]

        # X loads first on the SP ring (startup critical path); pair-0 W
        # slices after the first four tiles so QK0 can interleave with X^T
        xfs = []
        for i in range(NS):
            xf = xstage.tile([P, D], f32, name=f"xf{i}", tag="xf", bufs=4)
            nc.sync.dma_start(
                out=xf[:, 0:QC], in_=x_d[i * P:(i + 1) * P, 0:QC]
            )
            nc.sync.dma_start(
                out=xf[:, QC:D], in_=x_d[i * P:(i + 1) * P, QC:D]
            )
            xfs.append(xf)

        # All ACT-ring DMA triggers fire BEFORE any ACT compute is queued
        # (strict FIFO): masks, pair-0/1 W slices, then W_v — a DMA queue
        # parallel to the X load on the SP ring.
        msks = []
        for st in range(NS):
            msk = maskp.tile([P, 1], f32, name=f"msk{st}", tag="msk")
            nc.scalar.dma_start(
                out=msk,
                in_=m_d[st * P:(st + 1) * P].rearrange("(p o) -> p o", o=1),
            )
            msks.append(msk)

        # X cast + PE transpose; PSUM unloads in groups of 4 chunks,
        # alternating DVE / ACT
        def emit_xt(i):
            xb = xstage.tile([P, D], bf16, name=f"xb{i}", tag="xb", bufs=1)
            for g in range(2):
                nc.vector.tensor_copy(
                    xb[:, g * QC:(g + 1) * QC], xfs[i][:, g * QC:(g + 1) * QC]
                )
                pst = psB.tile([P, 4 * P], bf16, name=f"px{i}_{g}", tag="psB")
                for c in range(4):
                    j = 4 * g + c
                    nc.tensor.transpose(
                        pst[:, c * P:(c + 1) * P], xb[:, j * P:(j + 1) * P], id_bf
                    )
                eng = nc.vector if (2 * i + g) % 2 else nc.scalar
                dst = xt[:, 4 * g:4 * g + 4, i * P:(i + 1) * P]
                srcp = pst.rearrange("p (c q) -> p c q", c=4)
                if eng is nc.vector:
                    eng.tensor_copy(dst, srcp)
                else:
                    eng.copy(dst, srcp)

        def qk_load_dma(hp, eng=None):
            # W_q/W_k column slice DMAs for this head pair (SP ring by
            # default; startup loads ride the ACT ring so they do not queue
            # behind the 4MB X load on the SP ring's DMA queue)
            wfs = []
            for t, base in enumerate((hp * P, D + hp * P)):
                wf = wstage.tile([P, NK, P], f32, name=f"wf{hp}_{t}", tag="wf")
                (eng or nc.sync).dma_start(
                    out=wf,
                    in_=w_d[:, base:base + P].rearrange("(kt p) c -> p kt c", p=P),
                )
                wfs.append(wf)
            return wfs

        def qk_cast(hp, wfs):
            # bf16 casts (DVE) — issued well after the DMA so the strict-FIFO
            # DVE queue never stalls on the DMA semaphore
            wbf = []
            for t, wf in enumerate(wfs):
                wb = wqkp.tile([P, NK, P], bf16, name=f"wb{hp}_{t}", tag="wb")
                nc.vector.tensor_copy(wb, wf)
                wbf.append(wb)
            return wbf

        def qk_load(hp):
            return qk_cast(hp, qk_load_dma(hp))

        # pair-0/1 W DMAs on the ACT ring (queued right behind the tiny
        # mask DMAs: W0 lands ~t5us, long before QK0 needs it)
        wfs0 = qk_load_dma(0, eng=nc.scalar)

        # W_v full load on the SP ring behind X (lands ~t26us, before the
        # pair-0 V' matmuls need it)
        wvb = wvp.tile([P, NK, D], bf16, name="wvb")
        wvfs = []
        for q in range(4):
            wvf = wvstage.tile([P, 2, D], f32, name=f"wvf{q}", tag="wvf")
            nc.sync.dma_start(
                out=wvf,
                in_=w_d[2 * q * P:(2 * q + 2) * P, 2 * D:3 * D].rearrange(
                    "(kt p) c -> p kt c", p=P
                ),
            )
            wvfs.append(wvf)

        # exp(mask) per seq tile — the first ACT compute in the queue
        em = []
        for st in range(NS):
            emt = maskp.tile([P, 1], f32, name=f"em{st}", tag="em")
            nc.scalar.activation(emt, msks[st], Exp)
            em.append(emt)

        # X^T for the first four tiles, then the pair-0 W cast
        for i in range(4):
            emit_xt(i)
        wbf0 = qk_cast(0, wfs0)
        # pair-1 W triggers AFTER the pair-0 cast: their wstage-slot wait
        # (ring of 3, slot freed by that cast) must not block the ACT
        # stream's em/vcols/X^T-unload ops behind it
        wfs1 = qk_load_dma(1, eng=nc.scalar)

        # V' denominator columns = exp(mask) per key row (Pool engine)
        for st in range(NS):
            vcols = v_sb[st].rearrange("p (h c) -> p h c", h=H)[:, :, DH]
            nc.scalar.mul(vcols, ones16, em[st])

        def qk_chunks(hp, wbf):
            # QK as 8 four-matmul chunks into 1-bank psB tiles; each
            # (wsel, n) group is two chunks + a DVE unload, interleavable
            # between score units
            qt_t = qktp.tile([P, S], bf16, name=f"qt{hp}", tag="qt")
            kt_t = qktp.tile([P, S], bf16, name=f"kt{hp}", tag="kt")
            chunks = []
            for wsel, dest in ((1, kt_t), (0, qt_t)):
                for n in range(NQ):
                    cell = {}

                    def c0(cell=cell, wsel=wsel, n=n):
                        ps = psB.tile(
                            [P, QC], f32, name=f"pq{hp}_{wsel}_{n}", tag="psB"
                        )
                        cell["ps"] = ps
                        for k in range(4):
                            nc.tensor.matmul(
                                ps,
                                wbf[wsel][:, k, :],
                                xt[:, k, n * QC:(n + 1) * QC],
                                start=(k == 0),
                                stop=False,
                            )

                    def c1(cell=cell, wsel=wsel, n=n, dest=dest):
                        ps = cell["ps"]
                        for k in range(4, NK):
                            nc.tensor.matmul(
                                ps,
                                wbf[wsel][:, k, :],
                                xt[:, k, n * QC:(n + 1) * QC],
                                start=False,
                                stop=(k == NK - 1),
                            )
                        nc.vector.tensor_copy(
                            dest[:, n * QC:(n + 1) * QC], ps
                        )

                    chunks.append(c0)
                    chunks.append(c1)
            return qt_t, kt_t, chunks

        def emit_v_st(st):
            # V' [S, H*(Dh+2) padded]: stationary X^T chunks, 512-wide W_v;
            # per-key exp(mask) row scaling on the Pool engine
            for half in range(2):
                ps = psB.tile([P, QC], f32, name=f"pv{st}_{half}", tag="psB")
                for k in range(NK):
                    nc.tensor.matmul(
                        ps,
                        xt[:, k, st * P:(st + 1) * P],
                        wvb[:, k, half * QC:(half + 1) * QC],
                        start=(k == 0),
                        stop=(k == NK - 1),
                    )
                vdst = v_sb[st].rearrange("p (h c) -> p h c", h=H)[
                    :, half * 8:(half + 1) * 8, 0:DH
                ]
                vsrc = ps.rearrange("p (h c) -> p h c", h=8)
                nc.scalar.mul(vdst, vsrc, em[st])

        def scores_tiles(h):
            return [
                esp.tile([P, S], bf16, name=f"e{h}_{k}", tag="es") for k in range(NK)
            ]

        def emit_scores_unit(hp, k, esa, esb_, qt_t, kt_t):
            # one k-chunk of BOTH heads of the pair. Each psA tile holds one
            # qn half of BOTH heads ([a | b]), so all four matmuls of a unit
            # become schedule-ready together (the previous unit's exps free
            # both halves at once) and the a/b matmuls run CONCURRENTLY in
            # disjoint PE row groups (a: Q^T/K^T rows 0:64 / row group h0,
            # b: rows 64:128 / h64).
            for qn in range(NQ):
                ps = psA.tile([P, S], f32, name=f"s{hp}_{k}_{qn}", tag="psA")
                nc.tensor.matmul(
                    ps[:, 0:QC],
                    kt_t[0:DH, k * P:(k + 1) * P],
                    qt_t[0:DH, qn * QC:(qn + 1) * QC],
                    start=True,
                    stop=True,
                )
                nc.tensor.matmul(
                    ps[:, QC:S],
                    kt_t[DH:P, k * P:(k + 1) * P],
                    qt_t[DH:P, qn * QC:(qn + 1) * QC],
                    start=True,
                    stop=True,
                )
                nc.scalar.activation(
                    esa[k][:, qn * QC:(qn + 1) * QC], ps[:, 0:QC],
                    Exp, scale=SCALE,
                )
                nc.scalar.activation(
                    esb_[k][:, qn * QC:(qn + 1) * QC], ps[:, QC:S],
                    Exp, scale=SCALE,
                )

        def emit_pv_half(h, esb_, qn):
            # ctx'^T [65, S_q] = V'.T @ expS^T; SBUF bf16 copy (DVE)
            psc = psC.tile([VW, QC], f32, name=f"c{h}_{qn}", tag="psC")
            for k in range(NK):
                nc.tensor.matmul(
                    psc,
                    v_sb[k][:, h * VP:h * VP + VW],
                    esb_[k][:, qn * QC:(qn + 1) * QC],
                    start=(k == 0),
                    stop=(k == NK - 1),
                )
            ct = ctp.tile([VW, QC], bf16, name=f"ct{h}_{qn}", tag="ct")
            nc.vector.tensor_copy(ct, psc)
            return ct

        def emit_ctxt_qn(h, ct, qn, split_muls=False):
            # 4 bf16 PE transposes per PSUM tile back to [S_q, 65];
            # one strided reciprocal per 4 denominators; normalize on DVE
            VW2 = VW + 1  # 66: keeps each chunk's PSUM byte offset 4B-aligned
            pst = psB.tile([P, 4 * VW2], bf16, name=f"pt{h}_{qn}", tag="psB")
            for qs in range(QC // P):
                nc.tensor.transpose(
                    pst[:, qs * VW2:qs * VW2 + VW],
                    ct[:, qs * P:(qs + 1) * P],
                    id_bf[0:VW, 0:VW],
                )
            rec = smallp.tile([P, 4], f32, name=f"r{h}_{qn}", tag="rec")
            pst4 = pst.rearrange("p (c w) -> p c w", w=VW2)
            nc.vector.reciprocal(rec, pst4[:, 0:4, DH])
            for qs in range(QC // P):
                qi = qn * (QC // P) + qs
                if split_muls and qs % 2:
                    nc.scalar.mul(
                        ctx_sb[qi][:, h * DH:(h + 1) * DH],
                        pst[:, qs * VW2:qs * VW2 + DH],
                        rec[:, qs:qs + 1],
                    )
                else:
                    nc.vector.tensor_scalar_mul(
                        ctx_sb[qi][:, h * DH:(h + 1) * DH],
                        pst[:, qs * VW2:qs * VW2 + DH],
                        rec[:, qs:qs + 1],
                    )

        def emit_out_cols(c0, c1, st0=0, st1=NS, eng=None):
            # columns [c0, c1) final for rows [st0*P, st1*P): one 3D DMA.
            # Rides the Pool SWDGE queue by default so output transfers
            # never queue behind the W loads on the SP ring's queue.
            (eng or nc.gpsimd).dma_start(
                out=o_d[st0 * P:st1 * P, c0:c1].rearrange(
                    "(st p) c -> p st c", p=P
                ),
                in_=ctx_all[:, st0:st1, c0:c1],
            )

        # ---- startup: X^T with QK0 via the psB chunk mechanism (NOT the
        # old monolithic psA tiles, which held both psA ring slots until
        # their full-width unloads and so gated pair-0's first score
        # matmuls on X tiles 4-7; with psA free and kt0/qt0 landing
        # per-half, the scheduler hoists unit-0/1 qn0 scores into the
        # X-feed gaps). Weave order keeps every psB slot's closer ahead
        # of its reuse, and the n=1 chunks strictly after emit_xt(7)
        # (they read tile-7 columns — circular with px7 otherwise).
        qt0, kt0, chunks0 = qk_chunks(0, wbf0)
        chunks0[0]()
        chunks0[1]()
        emit_xt(4)
        chunks0[4]()
        chunks0[5]()
        emit_xt(5)
        emit_xt(6)
        emit_xt(7)
        chunks0[2]()
        chunks0[3]()
        chunks0[6]()
        chunks0[7]()

        # pair-1 W casts first (data already landed), then W_v casts
        wbf1 = []
        for t, wf in enumerate(wfs1):
            wb = wqkp.tile([P, NK, P], bf16, name=f"wb1_{t}", tag="wb")
            nc.vector.tensor_copy(wb, wf)
            wbf1.append(wb)
        for q in range(4):
            nc.vector.tensor_copy(wvb[:, 2 * q:2 * q + 2, :], wvfs[q])

        # ---- pair 0: interleaved scores(0,1) with QK1 chunks + V' fillers
        # (chunk pairs c0+c1 emitted whole so the psB ring alternates
        # cleanly with the V' tiles — no cross-tile open-group interleave)
        qt1, kt1, chunks1 = qk_chunks(1, wbf1)
        wf2 = qk_load_dma(2)
        es0, es1 = scores_tiles(0), scores_tiles(1)
        for k in range(NK):
            emit_scores_unit(0, k, es0, es1, qt0, kt0)
            if k % 2 == 1:
                chunks1[k - 1]()
                chunks1[k]()
            emit_v_st(k)
        wbf_store = {2: qk_cast(2, wf2)}

        # ---- steady-state pairs 1..7. es sets are a 3-deep ring: head 2hp
        # reuses head 2hp-3's tiles (PV'd mid pair hp-1) and head 2hp+1
        # reuses 2hp-2's, whose PV must therefore run BEFORE this pair's
        # units (the between-block).
        es_sets = {0: es0, 1: es1}
        ct_store = {}
        wf_store = {}
        qt_cur, kt_cur = qt1, kt1
        for hp in range(1, NHP):
            h_even_prev = 2 * hp - 2      # even head of pair hp-1
            h_odd_prev = 2 * hp - 1       # odd head of pair hp-1
            h_odd_prev2 = 2 * hp - 3      # odd head of pair hp-2
            qt_t, kt_t = qt_cur, kt_cur

            # between-block: PV of the even head of the previous pair
            es_even = es_sets.pop(h_even_prev)
            cts_prev = [emit_pv_half(h_even_prev, es_even, 0),
                        emit_pv_half(h_even_prev, es_even, 1)]

            if hp + 1 < NHP:
                qtn, ktn, chunks = qk_chunks(hp + 1, wbf_store.pop(hp + 1))
            else:
                qtn = ktn = None
                chunks = [None] * 8
            if hp + 2 < NHP:
                wf_store[hp + 2] = qk_load_dma(hp + 2)

            es_odd = es_sets.pop(h_odd_prev)
            es_a = scores_tiles(2 * hp)
            es_b = scores_tiles(2 * hp + 1)

            def unit(k, *fillers):
                emit_scores_unit(hp, k, es_a, es_b, qt_t, kt_t)
                for f in fillers:
                    if f is not None:
                        f()

            if hp >= 2:
                ct_p2 = ct_store.pop(h_odd_prev2)
            if hp < NHP - 1:
                unit(0, chunks[0],
                     None if hp < 2 else lambda: emit_ctxt_qn(
                         h_odd_prev2, ct_p2[0], 0))
                unit(1, chunks[1],
                     None if hp < 2 else lambda: emit_ctxt_qn(
                         h_odd_prev2, ct_p2[1], 1))
                unit(2, lambda: ct_store.setdefault(h_odd_prev, []).append(
                    emit_pv_half(h_odd_prev, es_odd, 0)), chunks[2])
                unit(3, chunks[3],
                     lambda: emit_ctxt_qn(h_even_prev, cts_prev[0], 0))
                unit(4, chunks[4],
                     lambda: emit_ctxt_qn(h_even_prev, cts_prev[1], 1))
                unit(5, lambda: ct_store[h_odd_prev].append(
                    emit_pv_half(h_odd_prev, es_odd, 1)), chunks[5])
                unit(6, chunks[6])
                unit(7, chunks[7])
            else:
                # last pair: no QK chunks
                unit(0, lambda: emit_ctxt_qn(h_odd_prev2, ct_p2[0], 0))
                unit(1, lambda: emit_ctxt_qn(h_odd_prev2, ct_p2[1], 1))
                unit(2, lambda: ct_store.setdefault(h_odd_prev, []).append(
                    emit_pv_half(h_odd_prev, es_odd, 0)))
                unit(3, lambda: emit_ctxt_qn(h_even_prev, cts_prev[0], 0))
                unit(4, lambda: emit_ctxt_qn(h_even_prev, cts_prev[1], 1))
                unit(5, lambda: ct_store[h_odd_prev].append(
                    emit_pv_half(h_odd_prev, es_odd, 1)))
                unit(6, None)
                unit(7, None)
            if hp + 2 < NHP:
                wbf_store[hp + 2] = qk_cast(hp + 2, wf_store.pop(hp + 2))
            es_sets[2 * hp] = es_a
            es_sets[2 * hp + 1] = es_b
            qt_cur, kt_cur = qtn, ktn
            if hp == 3:
                emit_out_cols(0, 5 * DH)
            elif hp == 5:
                emit_out_cols(5 * DH, 9 * DH)
            elif hp == 7:
                emit_out_cols(9 * DH, 13 * DH)

        # ---- tail: heads 13 (ctxT), 14, 15 (PV + ctxT). The last columns
        # go out as two row-half DMAs on different rings (parallel queues),
        # the first fired as soon as the qn0 muls land.
        es14 = es_sets.pop(14)
        es15 = es_sets.pop(15)
        ct13 = ct_store.pop(13)
        ct14_0 = emit_pv_half(14, es14, 0)
        emit_ctxt_qn(13, ct13[0], 0)
        ct14_1 = emit_pv_half(14, es14, 1)
        emit_ctxt_qn(14, ct14_0, 0, split_muls=True)
        ct15_0 = emit_pv_half(15, es15, 0)
        emit_ctxt_qn(13, ct13[1], 1)
        ct15_1 = emit_pv_half(15, es15, 1)
        emit_ctxt_qn(15, ct15_0, 0, split_muls=True)
        emit_out_cols(13 * DH, 16 * DH, 0, NS // 2, eng=nc.gpsimd)
        emit_ctxt_qn(14, ct14_1, 1, split_muls=True)
        emit_ctxt_qn(15, ct15_1, 1, split_muls=True)
        emit_out_cols(13 * DH, 16 * DH, NS // 2, NS, eng=nc.scalar)

    nc.finalize()
    return nc


def _get_nc():
    if "nc" not in _NC_CACHE:
        _NC_CACHE["nc"] = _build_nc()
    return _NC_CACHE["nc"]


def _run(hidden_states, attention_mask, qkv_weight, trace=False, **trace_kw):
    from concourse.bass_utils import run_bass_kernel_spmd

    nc = _get_nc()
    hidden = np.ascontiguousarray(np.asarray(hidden_states, dtype=np.float32))
    mask = np.ascontiguousarray(
        np.asarray(attention_mask, dtype=np.float32).reshape(B, S)
    )
    w = np.ascontiguousarray(np.asarray(qkv_weight, dtype=np.float32))
    in_maps = [
        {"x": hidden[b], "w": w, "m": mask[b]} for b in range(B)
    ]
    res = run_bass_kernel_spmd(nc, in_maps, list(range(B)), trace=trace, **trace_kw)
    out = np.stack([np.asarray(res.results[b]["o"]) for b in range(B)], axis=0)
    return out.astype(np.float32), res


def kernel(hidden_states, attention_mask, qkv_weight):
    out, _ = _run(hidden_states, attention_mask, qkv_weight, trace=False)
    return out


if __name__ == "__main__":
    _build_nc()
    print("build ok")
